# revision 11
# baseline (speedup 1.0000x reference)
"""Trainium2 Bass kernel for nn_MoEFFNBlock (B=2,S=2048,D=1024,H=2048,E=8,K=2).

Strategy (expert-parallel, 8 cores), v2:
  host: fp32 router (matches the jax reference selection), per expert
        split its tokens into a bf16 segment (rank-1 picks plus rank-2
        picks with combine weight >= THETA) and an fp8 segment (rank-2
        picks with weight < THETA). Normalized combine weights are
        folded into the up-projection on device.
  core e: SwiGLU FFN for expert e over both segments: stage 1+2 in
        bf16 matmuls for segment A, fp8 e4m3 DoubleRow matmuls (2x PE
        rate) for segment B; stage 3 (down-proj) in bf16 over the
        concatenated segments. Plus a (tokens/4, H/2) shard of the
        shared expert: core c handles token slice c%4 and H-half c//4.
  host: scatter-add segment outputs, sum the two H-half partials of
        the shared expert.

All matmul operands are pre-tiled on host so every DMA is a large
contiguous transfer. Chunk widths are kept >= 256 columns so the
per-matmul LDWEIGHTS (97ns bf16 / 134ns fp8-DoubleRow) stays hidden
under the matmul stream.
"""

import json
import math

import numpy as np

_B, _S, _D, _H, _E = 2, 2048, 1024, 2048, 8
_T = _B * _S
_P = 128
_NC = 8
_DK = _D // _P  # 8 contraction tiles over D
_HT = _H // _P  # 16 tiles over H
_HH = _H // 2  # shared-expert H columns per core (h-half)
_SHT = _HH // _P  # 8 h-tiles per core for the shared expert
_TS = _T // 4  # shared-expert token slice per core (1024)
_TC = 512  # shared-expert token chunk
_THETA = 0.45  # rank-2 combine-weight threshold for the fp8 segment

_TPB_ENGINES = {"PE", "Activation", "DVE", "Pool", "SP"}


def _split_waits(bir_bytes: bytes) -> bytes:
    """walrus in this container accepts only one sync-wait per TPB
    instruction; Tile's tail drain carries several. Hoist extras onto
    NoOps that run just before the instruction on the same engine."""
    m = json.loads(bir_bytes)
    ctr = 0
    for f in m["functions"]:
        blocks = f["blocks"]
        items = blocks.items() if isinstance(blocks, dict) else enumerate(blocks)
        for _bname, bb in items:
            new_insts = []
            for inst in bb["instructions"]:
                si = inst.get("sync_info") or {}
                ow = si.get("on_wait") or []
                if len(ow) > 1 and inst.get("engine") in _TPB_ENGINES:
                    for w in ow[:-1]:
                        ctr += 1
                        nop = {
                            "name": f"I-waitsplit-{ctr}",
                            "engine": inst["engine"],
                            "opcode": "NoOp",
                            "ins": [],
                            "outs": [],
                            "sync_info": {"on_wait": [w], "on_update": []},
                        }
                        if "debug" in inst:
                            nop["debug"] = inst["debug"]
                        new_insts.append(nop)
                    si["on_wait"] = [ow[-1]]
                new_insts.append(inst)
            bb["instructions"] = new_insts
    return json.dumps(m).encode()


def _chunks(C):
    """Column chunks, each 256..512 wide. C must be a multiple of 64,
    C >= 256."""
    assert C >= 256 and C % 64 == 0
    out = []
    rem = C
    while rem > 512:
        w = 512 if rem >= 768 else rem - 256
        out.append(w)
        rem -= w
    out.append(rem)
    ccs, o = [], 0
    for w in out:
        assert 256 <= w <= 512
        ccs.append((o, w))
        o += w
    assert o == C
    return ccs


def _build(C1, C2):
    import concourse.bass as bass
    import concourse.mybir as mybir
    import concourse.tile as tile

    f32 = mybir.dt.float32
    bf16 = mybir.dt.bfloat16
    f8e4 = mybir.dt.float8e4
    Silu = mybir.ActivationFunctionType.Silu
    mult = mybir.AluOpType.mult
    DR = mybir.MatmulPerfMode.DoubleRow

    C = C1 + C2

    nc = bass.Bass(trn_type="TRN2")
    xe1 = nc.dram_tensor("xe1", [_P, _DK, C1], bf16, kind="ExternalInput")
    xe2 = nc.dram_tensor("xe2", [_P, _DK, C2], f8e4, kind="ExternalInput")
    cw = nc.dram_tensor("cw", [_P, C], f32, kind="ExternalInput")
    scal = nc.dram_tensor("scal", [_P, 1], f32, kind="ExternalInput")
    wg1 = nc.dram_tensor("wg1", [_HT, _P, _DK, _P], bf16, kind="ExternalInput")
    wu1 = nc.dram_tensor("wu1", [_HT, _P, _DK, _P], bf16, kind="ExternalInput")
    wg8 = nc.dram_tensor("wg8", [_HT, _P, _DK, _P], f8e4, kind="ExternalInput")
    wu8 = nc.dram_tensor("wu8", [_HT, _P, _DK, _P], f8e4, kind="ExternalInput")
    wd1 = nc.dram_tensor("wd1", [_DK, _P, _HT, _P], bf16, kind="ExternalInput")
    xt = nc.dram_tensor("xt", [_P, _DK, _TS], bf16, kind="ExternalInput")
    sgh = nc.dram_tensor("sgh", [_P, _DK, _HH], bf16, kind="ExternalInput")
    suh = nc.dram_tensor("suh", [_P, _DK, _HH], bf16, kind="ExternalInput")
    sdh = nc.dram_tensor("sdh", [_P, _SHT, _D], bf16, kind="ExternalInput")
    rout = nc.dram_tensor("rout", [_DK, _P, C], bf16, kind="ExternalOutput")
    shout = nc.dram_tensor("shout", [_DK, _P, _TS], bf16, kind="ExternalOutput")

    ccs1 = _chunks(C1)
    ccsF = _chunks(C)

    with tile.TileContext(nc) as tc:
        with (
            tc.tile_pool(name="tmp", bufs=2) as tmp,
            tc.tile_pool(name="ps", bufs=2, space="PSUM") as psp,
            tc.tile_pool(name="bigS", bufs=1) as bigS,
            tc.tile_pool(name="cwg", bufs=1) as cwg,
            tc.tile_pool(name="strDW", bufs=3) as strDW,
        ):
            # PE warmup: dummy matmuls so the PE p-state ramps while the
            # initial DMAs are in flight.
            wtile32 = cwg.tile([_P, 512], f32, name="wtile32")
            nc.vector.memset(wtile32[:], 0.0)
            wtile = cwg.tile([_P, 512], bf16, name="wtile")
            nc.vector.tensor_copy(wtile[:], wtile32[:])
            wps = psp.tile([_P, 512], f32, tag="out", name="ops", bufs=4)
            for i in range(24):
                nc.tensor.matmul(
                    wps[:],
                    wtile[:, :_P],
                    wtile[:],
                    start=(i == 0),
                    stop=(i == 23),
                )

            cw_sb = cwg.tile([_P, C], f32, name="cw_sb")
            scal_sb = cwg.tile([_P, 1], f32, name="scal_sb")
            g_sb = cwg.tile([_P, _HT, C], bf16, name="g_sb")
            sg_sb = bigS.tile([_P, _DK, _HH], bf16, name="sg_sb")
            su_sb = bigS.tile([_P, _DK, _HH], bf16, name="su_sb")
            sd_sb = bigS.tile([_P, _SHT, _D], bf16, name="sd_sb")

            # ---------- segment A (bf16) stage 1 -------------------------
            with (
                tc.tile_pool(name="poolXE", bufs=1) as poolXE,
                tc.tile_pool(name="strGU", bufs=3) as strGU,
                tc.tile_pool(name="strGU8", bufs=3) as strGU8,
            ):
                # First-needed data first: xe1 chunk 0 and the first weight
                # tiles, then the rest.
                xe1_sb = poolXE.tile([_P, _DK, C1], bf16, name="xe1_sb")
                c0_, cn_ = ccs1[0]
                nc.sync.dma_start(
                    xe1_sb[:, :, c0_ : c0_ + cn_], xe1.ap()[:, :, c0_ : c0_ + cn_]
                )
                wgu_tiles = []
                for ht in range(2):
                    wg_t = strGU.tile([_P, _DK, _P], bf16, tag="wg", name="wg_t")
                    nc.sync.dma_start(wg_t[:], wg1.ap()[ht])
                    wu_t = strGU.tile([_P, _DK, _P], bf16, tag="wu", name="wu_t")
                    nc.sync.dma_start(wu_t[:], wu1.ap()[ht])
                    wgu_tiles.append((wg_t, wu_t))
                for c0_, cn_ in ccs1[1:]:
                    nc.gpsimd.dma_start(
                        xe1_sb[:, :, c0_ : c0_ + cn_], xe1.ap()[:, :, c0_ : c0_ + cn_]
                    )
                nc.gpsimd.dma_start(cw_sb[:], cw.ap())
                nc.gpsimd.dma_start(scal_sb[:], scal.ap())
                xe2_sb = poolXE.tile([_P, _DK, C2], f8e4, name="xe2_sb")
                nc.gpsimd.dma_start(xe2_sb[:], xe2.ap())

                for ht in range(_HT):
                    if ht < 2:
                        wg_t, wu_t = wgu_tiles[ht]
                    else:
                        wg_t = strGU.tile([_P, _DK, _P], bf16, tag="wg", name="wg_t")
                        nc.sync.dma_start(wg_t[:], wg1.ap()[ht])
                        wu_t = strGU.tile([_P, _DK, _P], bf16, tag="wu", name="wu_t")
                        nc.sync.dma_start(wu_t[:], wu1.ap()[ht])
                    for c0, cn in ccs1:
                        h1 = psp.tile([_P, 512], f32, tag="h1", name="h1ps")[:, :cn]
                        for k in range(_DK):
                            nc.tensor.matmul(
                                h1,
                                wg_t[:, k],
                                xe1_sb[:, k, c0 : c0 + cn],
                                start=(k == 0),
                                stop=(k == _DK - 1),
                            )
                        h2 = psp.tile([_P, 512], f32, tag="h2", name="h2ps")[:, :cn]
                        for k in range(_DK):
                            nc.tensor.matmul(
                                h2,
                                wu_t[:, k],
                                xe1_sb[:, k, c0 : c0 + cn],
                                start=(k == 0),
                                stop=(k == _DK - 1),
                            )
                        sl = tmp.tile([_P, 512], f32, tag="sl", name="sl_sb", bufs=3)[
                            :, :cn
                        ]
                        nc.scalar.activation(sl, h1, Silu)
                        t2 = tmp.tile([_P, 512], f32, tag="t2", name="t2_sb", bufs=3)[
                            :, :cn
                        ]
                        nc.vector.tensor_tensor(t2, h2, cw_sb[:, c0 : c0 + cn], mult)
                        nc.vector.tensor_tensor(
                            g_sb[:, ht, c0 : c0 + cn], sl, t2, mult
                        )

                # ---------- segment B (fp8 DoubleRow) stage 1 ------------
                # Shared-expert weights load on the Pool DMA ring so they
                # never queue ahead of the latency-critical weight streams.
                nc.gpsimd.dma_start(sg_sb[:], sgh.ap())
                nc.gpsimd.dma_start(su_sb[:], suh.ap())
                nc.gpsimd.dma_start(sd_sb[:], sdh.ap())

                for ht in range(_HT):
                    wg8_t = strGU8.tile([_P, _DK, _P], f8e4, tag="wg8", name="wg8_t")
                    nc.sync.dma_start(wg8_t[:], wg8.ap()[ht])
                    wu8_t = strGU8.tile([_P, _DK, _P], f8e4, tag="wu8", name="wu8_t")
                    nc.sync.dma_start(wu8_t[:], wu8.ap()[ht])
                    h1 = psp.tile([_P, 512], f32, tag="h1", name="h1ps")[:, :C2]
                    for k in range(_DK // 2):
                        nc.tensor.matmul(
                            h1,
                            wg8_t[:, 2 * k : 2 * k + 2],
                            xe2_sb[:, 2 * k : 2 * k + 2],
                            start=(k == 0),
                            stop=(k == _DK // 2 - 1),
                            perf_mode=DR,
                        )
                    h2 = psp.tile([_P, 512], f32, tag="h2", name="h2ps")[:, :C2]
                    for k in range(_DK // 2):
                        nc.tensor.matmul(
                            h2,
                            wu8_t[:, 2 * k : 2 * k + 2],
                            xe2_sb[:, 2 * k : 2 * k + 2],
                            start=(k == 0),
                            stop=(k == _DK // 2 - 1),
                            perf_mode=DR,
                        )
                    sl = tmp.tile([_P, 512], f32, tag="sl", name="sl_sb", bufs=3)[
                        :, :C2
                    ]
                    nc.scalar.activation(sl, h1, Silu, scale=scal_sb[:, 0:1])
                    t2 = tmp.tile([_P, 512], f32, tag="t2", name="t2_sb", bufs=3)[
                        :, :C2
                    ]
                    nc.vector.tensor_tensor(t2, h2, cw_sb[:, C1:C], mult)
                    nc.vector.tensor_tensor(g_sb[:, ht, C1:C], sl, t2, mult)

            # ---------- stage 2 (down-proj, both segments) ---------------
            with (
                tc.tile_pool(name="strS", bufs=2) as strS,
                tc.tile_pool(name="gsp", bufs=2) as gsp,
            ):
                # Prefetch both shared-expert token chunks on the Pool ring.
                xt_tiles = []
                for t0 in range(0, _TS, _TC):
                    xt_sb = strS.tile([_P, _DK, _TC], bf16, tag="xt", name="xt_sb")
                    nc.gpsimd.dma_start(xt_sb[:], xt.ap()[:, :, t0 : t0 + _TC])
                    xt_tiles.append(xt_sb)

                for dt_i in range(_DK):
                    wd_t = strDW.tile([_P, _HT, _P], bf16, tag="wd", name="wd_t")
                    nc.sync.dma_start(wd_t[:], wd1.ap()[dt_i])
                    for c0, cn in ccsF:
                        ops = psp.tile(
                            [_P, 512], f32, tag="out", name="ops", bufs=4
                        )[:, :cn]
                        for k in range(_HT):
                            nc.tensor.matmul(
                                ops,
                                wd_t[:, k],
                                g_sb[:, k, c0 : c0 + cn],
                                start=(k == 0),
                                stop=(k == _HT - 1),
                            )
                        ro = tmp.tile(
                            [_P, 512], bf16, tag="ro", name="ro_sb", bufs=6
                        )[:, :cn]
                        if dt_i % 2:
                            nc.scalar.copy(ro, ops)
                        else:
                            nc.vector.tensor_copy(ro, ops)
                        nc.sync.dma_start(rout.ap()[dt_i][:, c0 : c0 + cn], ro)

                # ---------- shared expert, software-pipelined ------------
                def s_hstage(t0, xt_sb):
                    gs = gsp.tile([_P, _SHT, _TC], bf16, tag="gs", name="gs_sb")
                    for hs in range(_SHT):
                        h1 = psp.tile([_P, 512], f32, tag="h1", name="h1ps")
                        for k in range(_DK):
                            nc.tensor.matmul(
                                h1,
                                sg_sb[:, k, hs * _P : (hs + 1) * _P],
                                xt_sb[:, k],
                                start=(k == 0),
                                stop=(k == _DK - 1),
                            )
                        h2 = psp.tile([_P, 512], f32, tag="h2", name="h2ps")
                        for k in range(_DK):
                            nc.tensor.matmul(
                                h2,
                                su_sb[:, k, hs * _P : (hs + 1) * _P],
                                xt_sb[:, k],
                                start=(k == 0),
                                stop=(k == _DK - 1),
                            )
                        sl = tmp.tile([_P, 512], f32, tag="sl", name="sl_sb", bufs=3)
                        nc.scalar.activation(sl, h1, Silu)
                        nc.vector.tensor_tensor(gs[:, hs], sl, h2, mult)
                    return gs

                def s_dstage(t0, gs):
                    for dt_i in range(_DK):
                        ops = psp.tile([_P, 512], f32, tag="out", name="ops", bufs=4)
                        for k in range(_SHT):
                            nc.tensor.matmul(
                                ops,
                                sd_sb[:, k, dt_i * _P : (dt_i + 1) * _P],
                                gs[:, k],
                                start=(k == 0),
                                stop=(k == _SHT - 1),
                            )
                        so = tmp.tile(
                            [_P, 512], bf16, tag="ro", name="ro_sb", bufs=6
                        )
                        if dt_i % 2:
                            nc.scalar.copy(so, ops)
                        else:
                            nc.vector.tensor_copy(so, ops)
                        nc.sync.dma_start(shout.ap()[dt_i][:, t0 : t0 + _TC], so)

                prev = None
                for ci, t0 in enumerate(range(0, _TS, _TC)):
                    gs = s_hstage(t0, xt_tiles[ci])
                    if prev is not None:
                        s_dstage(prev[0], prev[1])
                    prev = (t0, gs)
                s_dstage(prev[0], prev[1])

    orig = nc.to_json_bytes
    nc.to_json_bytes = lambda: _split_waits(orig())
    return nc


def _route(xf, w_router):
    """fp32 router matching the jax reference: softmax over logits, top-2
    (selection identical to jax.lax.top_k for non-tied logits), weights
    renormalized over the selected pair."""
    logits = xf @ w_router.T.astype(np.float32)
    m = logits.max(-1, keepdims=True)
    p = np.exp(logits - m)
    p /= p.sum(-1, keepdims=True)
    i1 = p.argmax(-1)
    p2 = p.copy()
    p2[np.arange(p.shape[0]), i1] = -1.0
    i2 = p2.argmax(-1)
    w1 = p[np.arange(p.shape[0]), i1]
    w2 = p[np.arange(p.shape[0]), i2]
    s = w1 + w2
    return i1, i2, (w1 / s).astype(np.float32), (w2 / s).astype(np.float32)


def _tile_kxm(a2d, kouter, dtype):
    """[K, M] -> [128, K//128, M] with partition dim first."""
    k, mdim = a2d.shape
    assert k == kouter * _P
    return np.ascontiguousarray(
        a2d.reshape(kouter, _P, mdim).transpose(1, 0, 2)
    ).astype(dtype)


def _pad64(n):
    return max(256, int(math.ceil(n / 64.0)) * 64)


def _prepare(inputs):
    import ml_dtypes

    bf16 = ml_dtypes.bfloat16
    f8 = ml_dtypes.float8_e4m3

    x = np.asarray(inputs["x"], dtype=np.float32)
    w_router = np.asarray(inputs["w_router"], dtype=np.float32)
    Wg = np.asarray(inputs["Wg"], dtype=np.float32)
    Wu = np.asarray(inputs["Wu"], dtype=np.float32)
    Wd = np.asarray(inputs["Wd"], dtype=np.float32)
    sg = np.asarray(inputs["sg"], dtype=np.float32)
    su = np.asarray(inputs["su"], dtype=np.float32)
    sd = np.asarray(inputs["sd"], dtype=np.float32)

    xf = np.ascontiguousarray(x.reshape(_T, _D))
    i1, i2, w1, w2 = _route(xf, w_router)

    # Segment split per expert: A = rank-1 plus high-weight rank-2 (bf16),
    # B = low-weight rank-2 (fp8).
    idxA, cwA, idxB, cwB = [], [], [], []
    for e in range(_E):
        sel1 = i1 == e
        sel2 = i2 == e
        lo = sel2 & (w2 < _THETA)
        hi = sel1 | (sel2 & ~lo)
        ia = np.nonzero(hi)[0]
        wa = np.where(sel1[ia], w1[ia], w2[ia]).astype(np.float32)
        ib = np.nonzero(lo)[0]
        wb = w2[ib].astype(np.float32)
        idxA.append(ia)
        cwA.append(wa)
        idxB.append(ib)
        cwB.append(wb)
    C1 = _pad64(max(len(i) for i in idxA))
    C2 = max(384, _pad64(max(len(i) for i in idxB)))
    C = C1 + C2

    xt_f = np.ascontiguousarray(xf.T)  # [D, T]

    in_maps = []
    for e in range(_E):
        ia, wa = idxA[e], cwA[e]
        ib, wb = idxB[e], cwB[e]
        na, nb = len(ia), len(ib)

        xe1_h = np.zeros((_P, _DK, C1), bf16)
        if na:
            xe1_h[:, :, :na] = _tile_kxm(np.ascontiguousarray(xf[ia].T), _DK, bf16)

        # fp8 segment: per-tensor scales for x and the two up-projections.
        xb = xf[ib] if nb else np.zeros((1, _D), np.float32)
        sx = 240.0 / max(np.abs(xb).max(), 1e-30)
        swg = 240.0 / max(np.abs(Wg[e]).max(), 1e-30)
        swu = 240.0 / max(np.abs(Wu[e]).max(), 1e-30)
        xe2_h = np.zeros((_P, _DK, C2), f8)
        if nb:
            xq = np.clip(xf[ib] * sx, -240, 240)
            xe2_h[:, :, :nb] = _tile_kxm(np.ascontiguousarray(xq.T), _DK, f8)

        cw_h = np.zeros((_P, C), np.float32)
        cw_h[:, :na] = wa[None, :]
        cw_h[:, C1 : C1 + nb] = (wb / (sx * swu))[None, :]
        scal_h = np.full((_P, 1), 1.0 / (sx * swg), np.float32)

        wgT = np.ascontiguousarray(Wg[e].T)  # [D, H]
        wg_h = np.ascontiguousarray(
            wgT.reshape(_DK, _P, _HT, _P).transpose(2, 1, 0, 3)
        ).astype(bf16)
        wuT = np.ascontiguousarray(Wu[e].T)
        wu_h = np.ascontiguousarray(
            wuT.reshape(_DK, _P, _HT, _P).transpose(2, 1, 0, 3)
        ).astype(bf16)
        wg8_h = np.ascontiguousarray(
            np.clip(wgT * swg, -240, 240)
            .reshape(_DK, _P, _HT, _P)
            .transpose(2, 1, 0, 3)
        ).astype(f8)
        wu8_h = np.ascontiguousarray(
            np.clip(wuT * swu, -240, 240)
            .reshape(_DK, _P, _HT, _P)
            .transpose(2, 1, 0, 3)
        ).astype(f8)
        wdT = np.ascontiguousarray(Wd[e].T)  # [H, D]
        wd_h = np.ascontiguousarray(
            wdT.reshape(_HT, _P, _DK, _P).transpose(2, 1, 0, 3)
        ).astype(bf16)

        # shared expert shard: token slice e%4, H-half e//4
        tsl = slice((e % 4) * _TS, (e % 4 + 1) * _TS)
        hsl = slice((e // 4) * _HH, (e // 4 + 1) * _HH)
        xt_h = _tile_kxm(np.ascontiguousarray(xt_f[:, tsl]), _DK, bf16)
        sg_h = _tile_kxm(np.ascontiguousarray(sg[hsl].T), _DK, bf16)
        su_h = _tile_kxm(np.ascontiguousarray(su[hsl].T), _DK, bf16)
        sd_h = _tile_kxm(np.ascontiguousarray(sd[:, hsl].T), _SHT, bf16)

        in_maps.append(
            {
                "xe1": xe1_h,
                "xe2": xe2_h,
                "cw": cw_h,
                "scal": scal_h,
                "wg1": wg_h,
                "wu1": wu_h,
                "wg8": wg8_h,
                "wu8": wu8_h,
                "wd1": wd_h,
                "xt": xt_h,
                "sgh": sg_h,
                "suh": su_h,
                "sdh": sd_h,
            }
        )
    return in_maps, (idxA, idxB, C1, C2), (C1, C2)


def _combine(results, meta):
    idxA, idxB, C1, C2 = meta
    out = np.zeros((_D, _T), np.float32)
    for e in range(_E):
        ro = results[e]["rout"].astype(np.float32).reshape(_D, C1 + C2)
        sh = results[e]["shout"].astype(np.float32).reshape(_D, _TS)
        tsl = slice((e % 4) * _TS, (e % 4 + 1) * _TS)
        out[:, tsl] += sh
        if len(idxA[e]):
            out[:, idxA[e]] += ro[:, : len(idxA[e])]
        if len(idxB[e]):
            out[:, idxB[e]] += ro[:, C1 : C1 + len(idxB[e])]
    return np.ascontiguousarray(out.T).reshape(_B, _S, _D).astype(np.float32)


def kernel(**inputs):
    from concourse import bass_utils

    in_maps, meta, caps = _prepare(inputs)
    nc = _build(*caps)
    res = bass_utils.run_bass_kernel_spmd(nc, in_maps, core_ids=list(range(_NC)))
    return _combine(res.results, meta)


# revision 14
# speedup vs baseline: 1.0564x; 1.0564x over previous
"""Trainium2 Bass kernel for nn_MoEFFNBlock (B=2,S=2048,D=1024,H=2048,E=8,K=2).

Strategy (expert-parallel, 8 cores), v2:
  host: fp32 router (matches the jax reference selection), per expert
        split its tokens into a bf16 segment (rank-1 picks plus rank-2
        picks with combine weight >= THETA) and an fp8 segment (rank-2
        picks with weight < THETA). Normalized combine weights are
        folded into the up-projection on device.
  core e: SwiGLU FFN for expert e over both segments: stage 1+2 in
        bf16 matmuls for segment A, fp8 e4m3 DoubleRow matmuls (2x PE
        rate) for segment B; stage 3 (down-proj) in bf16 over the
        concatenated segments. Plus a (tokens/4, H/2) shard of the
        shared expert: core c handles token slice c%4 and H-half c//4.
  host: scatter-add segment outputs, sum the two H-half partials of
        the shared expert.

All matmul operands are pre-tiled on host so every DMA is a large
contiguous transfer. Chunk widths are kept >= 256 columns so the
per-matmul LDWEIGHTS (97ns bf16 / 134ns fp8-DoubleRow) stays hidden
under the matmul stream.
"""

import json
import math

import numpy as np

_B, _S, _D, _H, _E = 2, 2048, 1024, 2048, 8
_T = _B * _S
_P = 128
_NC = 8
_DK = _D // _P  # 8 contraction tiles over D
_HT = _H // _P  # 16 tiles over H
_HH = _H // 2  # shared-expert H columns per core (h-half)
_SHT = _HH // _P  # 8 h-tiles per core for the shared expert
_TS = _T // 4  # shared-expert token slice per core (1024)
_TC = 512  # shared-expert token chunk
_THETA = 0.45  # rank-2 combine-weight threshold for the fp8 segment

_TPB_ENGINES = {"PE", "Activation", "DVE", "Pool", "SP"}


def _split_waits(bir_bytes: bytes) -> bytes:
    """walrus in this container accepts only one sync-wait per TPB
    instruction; Tile's tail drain carries several. Hoist extras onto
    NoOps that run just before the instruction on the same engine."""
    m = json.loads(bir_bytes)
    ctr = 0
    for f in m["functions"]:
        blocks = f["blocks"]
        items = blocks.items() if isinstance(blocks, dict) else enumerate(blocks)
        for _bname, bb in items:
            new_insts = []
            for inst in bb["instructions"]:
                si = inst.get("sync_info") or {}
                ow = si.get("on_wait") or []
                if len(ow) > 1 and inst.get("engine") in _TPB_ENGINES:
                    for w in ow[:-1]:
                        ctr += 1
                        nop = {
                            "name": f"I-waitsplit-{ctr}",
                            "engine": inst["engine"],
                            "opcode": "NoOp",
                            "ins": [],
                            "outs": [],
                            "sync_info": {"on_wait": [w], "on_update": []},
                        }
                        if "debug" in inst:
                            nop["debug"] = inst["debug"]
                        new_insts.append(nop)
                    si["on_wait"] = [ow[-1]]
                new_insts.append(inst)
            bb["instructions"] = new_insts
    return json.dumps(m).encode()


def _chunks(C):
    """Column chunks, each 256..512 wide. C must be a multiple of 64,
    C >= 256."""
    assert C >= 256 and C % 64 == 0
    out = []
    rem = C
    while rem > 512:
        w = 512 if rem >= 768 else rem - 256
        out.append(w)
        rem -= w
    out.append(rem)
    ccs, o = [], 0
    for w in out:
        assert 256 <= w <= 512
        ccs.append((o, w))
        o += w
    assert o == C
    return ccs


def _build(C1, C2):
    import concourse.bass as bass
    import concourse.mybir as mybir
    import concourse.tile as tile

    f32 = mybir.dt.float32
    bf16 = mybir.dt.bfloat16
    f8e4 = mybir.dt.float8e4
    Silu = mybir.ActivationFunctionType.Silu
    mult = mybir.AluOpType.mult
    DR = mybir.MatmulPerfMode.DoubleRow

    C = C1 + C2

    nc = bass.Bass(trn_type="TRN2")
    xe1 = nc.dram_tensor("xe1", [_P, _DK, C1], bf16, kind="ExternalInput")
    xe2 = nc.dram_tensor("xe2", [_P, _DK, C2], f8e4, kind="ExternalInput")
    cw = nc.dram_tensor("cw", [_P, C], f32, kind="ExternalInput")
    scal = nc.dram_tensor("scal", [_P, 1], f32, kind="ExternalInput")
    wg1 = nc.dram_tensor("wg1", [_HT, _P, _DK, _P], bf16, kind="ExternalInput")
    wu1 = nc.dram_tensor("wu1", [_HT, _P, _DK, _P], bf16, kind="ExternalInput")
    wg8 = nc.dram_tensor("wg8", [_HT, _P, _DK, _P], f8e4, kind="ExternalInput")
    wu8 = nc.dram_tensor("wu8", [_HT, _P, _DK, _P], f8e4, kind="ExternalInput")
    wd1 = nc.dram_tensor("wd1", [_DK, _P, _HT, _P], bf16, kind="ExternalInput")
    xt = nc.dram_tensor("xt", [_P, _DK, _TS], bf16, kind="ExternalInput")
    sgh = nc.dram_tensor("sgh", [_P, _DK, _HH], bf16, kind="ExternalInput")
    suh = nc.dram_tensor("suh", [_P, _DK, _HH], bf16, kind="ExternalInput")
    sdh = nc.dram_tensor("sdh", [_P, _SHT, _D], bf16, kind="ExternalInput")
    rout = nc.dram_tensor("rout", [_DK, _P, C], bf16, kind="ExternalOutput")
    shout = nc.dram_tensor("shout", [_DK, _P, _TS], bf16, kind="ExternalOutput")

    ccs1 = _chunks(C1)
    ccsF = _chunks(C)

    with tile.TileContext(nc) as tc:
        with (
            tc.tile_pool(name="tmp", bufs=2) as tmp,
            tc.tile_pool(name="ps", bufs=2, space="PSUM") as psp,
            tc.tile_pool(name="bigS", bufs=1) as bigS,
            tc.tile_pool(name="cwg", bufs=1) as cwg,
            tc.tile_pool(name="strDW", bufs=3) as strDW,
        ):
            # PE warmup: dummy matmuls so the PE p-state ramps while the
            # initial DMAs are in flight.
            wtile32 = cwg.tile([_P, 512], f32, name="wtile32")
            nc.vector.memset(wtile32[:], 0.0)
            wtile = cwg.tile([_P, 512], bf16, name="wtile")
            nc.vector.tensor_copy(wtile[:], wtile32[:])
            wps = psp.tile([_P, 512], f32, tag="out", name="ops", bufs=4)
            for i in range(24):
                nc.tensor.matmul(
                    wps[:],
                    wtile[:, :_P],
                    wtile[:],
                    start=(i == 0),
                    stop=(i == 23),
                )

            cw_sb = cwg.tile([_P, C], f32, name="cw_sb")
            scal_sb = cwg.tile([_P, 1], f32, name="scal_sb")
            g_sb = cwg.tile([_P, _HT, C], bf16, name="g_sb")
            sg_sb = bigS.tile([_P, _DK, _HH], bf16, name="sg_sb")
            su_sb = bigS.tile([_P, _DK, _HH], bf16, name="su_sb")
            sd_sb = bigS.tile([_P, _SHT, _D], bf16, name="sd_sb")

            # ---------- segment A (bf16) stage 1 -------------------------
            with (
                tc.tile_pool(name="poolXE", bufs=1) as poolXE,
                tc.tile_pool(name="strGU", bufs=3) as strGU,
                tc.tile_pool(name="strGU8", bufs=3) as strGU8,
            ):
                # First-needed data first: xe1 chunk 0 and the first weight
                # tiles, then the rest.
                xe1_sb = poolXE.tile([_P, _DK, C1], bf16, name="xe1_sb")
                c0_, cn_ = ccs1[0]
                nc.sync.dma_start(
                    xe1_sb[:, :, c0_ : c0_ + cn_], xe1.ap()[:, :, c0_ : c0_ + cn_]
                )
                wgu_tiles = []
                for ht in range(2):
                    wg_t = strGU.tile([_P, _DK, _P], bf16, tag="wg", name="wg_t")
                    nc.sync.dma_start(wg_t[:], wg1.ap()[ht])
                    wu_t = strGU.tile([_P, _DK, _P], bf16, tag="wu", name="wu_t")
                    nc.sync.dma_start(wu_t[:], wu1.ap()[ht])
                    wgu_tiles.append((wg_t, wu_t))
                for c0_, cn_ in ccs1[1:]:
                    nc.sync.dma_start(
                        xe1_sb[:, :, c0_ : c0_ + cn_], xe1.ap()[:, :, c0_ : c0_ + cn_]
                    )
                nc.sync.dma_start(cw_sb[:], cw.ap())
                nc.sync.dma_start(scal_sb[:], scal.ap())
                xe2_sb = poolXE.tile([_P, _DK, C2], f8e4, name="xe2_sb")
                nc.sync.dma_start(xe2_sb[:], xe2.ap())

                for ht in range(_HT):
                    if ht < 2:
                        wg_t, wu_t = wgu_tiles[ht]
                    else:
                        wg_t = strGU.tile([_P, _DK, _P], bf16, tag="wg", name="wg_t")
                        nc.sync.dma_start(wg_t[:], wg1.ap()[ht])
                        wu_t = strGU.tile([_P, _DK, _P], bf16, tag="wu", name="wu_t")
                        nc.sync.dma_start(wu_t[:], wu1.ap()[ht])
                    for c0, cn in ccs1:
                        h1 = psp.tile([_P, 512], f32, tag="h1", name="h1ps")[:, :cn]
                        for k in range(_DK):
                            nc.tensor.matmul(
                                h1,
                                wg_t[:, k],
                                xe1_sb[:, k, c0 : c0 + cn],
                                start=(k == 0),
                                stop=(k == _DK - 1),
                            )
                        h2 = psp.tile([_P, 512], f32, tag="h2", name="h2ps")[:, :cn]
                        for k in range(_DK):
                            nc.tensor.matmul(
                                h2,
                                wu_t[:, k],
                                xe1_sb[:, k, c0 : c0 + cn],
                                start=(k == 0),
                                stop=(k == _DK - 1),
                            )
                        sl = tmp.tile([_P, 512], f32, tag="sl", name="sl_sb", bufs=3)[
                            :, :cn
                        ]
                        nc.scalar.activation(sl, h1, Silu)
                        t2 = tmp.tile([_P, 512], f32, tag="t2", name="t2_sb", bufs=3)[
                            :, :cn
                        ]
                        nc.vector.tensor_tensor(t2, h2, cw_sb[:, c0 : c0 + cn], mult)
                        nc.vector.tensor_tensor(
                            g_sb[:, ht, c0 : c0 + cn], sl, t2, mult
                        )

                # ---------- segment B (fp8 DoubleRow) stage 1 ------------
                for ht in range(_HT):
                    wg8_t = strGU8.tile([_P, _DK, _P], f8e4, tag="wg8", name="wg8_t")
                    nc.sync.dma_start(wg8_t[:], wg8.ap()[ht])
                    wu8_t = strGU8.tile([_P, _DK, _P], f8e4, tag="wu8", name="wu8_t")
                    nc.sync.dma_start(wu8_t[:], wu8.ap()[ht])
                    h1 = psp.tile([_P, 512], f32, tag="h1", name="h1ps")[:, :C2]
                    for k in range(_DK // 2):
                        nc.tensor.matmul(
                            h1,
                            wg8_t[:, 2 * k : 2 * k + 2],
                            xe2_sb[:, 2 * k : 2 * k + 2],
                            start=(k == 0),
                            stop=(k == _DK // 2 - 1),
                            perf_mode=DR,
                        )
                    h2 = psp.tile([_P, 512], f32, tag="h2", name="h2ps")[:, :C2]
                    for k in range(_DK // 2):
                        nc.tensor.matmul(
                            h2,
                            wu8_t[:, 2 * k : 2 * k + 2],
                            xe2_sb[:, 2 * k : 2 * k + 2],
                            start=(k == 0),
                            stop=(k == _DK // 2 - 1),
                            perf_mode=DR,
                        )
                    sl = tmp.tile([_P, 512], f32, tag="sl", name="sl_sb", bufs=3)[
                        :, :C2
                    ]
                    nc.scalar.activation(sl, h1, Silu, scale=scal_sb[:, 0:1])
                    t2 = tmp.tile([_P, 512], f32, tag="t2", name="t2_sb", bufs=3)[
                        :, :C2
                    ]
                    nc.vector.tensor_tensor(t2, h2, cw_sb[:, C1:C], mult)
                    nc.vector.tensor_tensor(g_sb[:, ht, C1:C], sl, t2, mult)

            # ---------- stage 2 (down-proj, both segments) ---------------
            with (
                tc.tile_pool(name="strS", bufs=2) as strS,
                tc.tile_pool(name="gsp", bufs=2) as gsp,
            ):
                xt_tiles = [
                    strS.tile([_P, _DK, _TC], bf16, tag="xt", name="xt_sb")
                    for _ in range(0, _TS, _TC)
                ]

                for dt_i in range(_DK):
                    wd_t = strDW.tile([_P, _HT, _P], bf16, tag="wd", name="wd_t")
                    nc.sync.dma_start(wd_t[:], wd1.ap()[dt_i])
                    # Interleave the bulk shared-expert loads between the
                    # down-proj weight tiles so neither starves the other.
                    if dt_i == 0:
                        nc.sync.dma_start(sg_sb[:], sgh.ap())
                    elif dt_i == 2:
                        nc.sync.dma_start(su_sb[:], suh.ap())
                    elif dt_i == 4:
                        nc.sync.dma_start(sd_sb[:], sdh.ap())
                    elif dt_i == 5:
                        nc.sync.dma_start(xt_tiles[0][:], xt.ap()[:, :, 0:_TC])
                    elif dt_i == 6:
                        nc.sync.dma_start(xt_tiles[1][:], xt.ap()[:, :, _TC : 2 * _TC])
                    for c0, cn in ccsF:
                        ops = psp.tile(
                            [_P, 512], f32, tag="out", name="ops", bufs=4
                        )[:, :cn]
                        for k in range(_HT):
                            nc.tensor.matmul(
                                ops,
                                wd_t[:, k],
                                g_sb[:, k, c0 : c0 + cn],
                                start=(k == 0),
                                stop=(k == _HT - 1),
                            )
                        ro = tmp.tile(
                            [_P, 512], bf16, tag="ro", name="ro_sb", bufs=6
                        )[:, :cn]
                        if dt_i % 2:
                            nc.scalar.copy(ro, ops)
                        else:
                            nc.vector.tensor_copy(ro, ops)
                        nc.sync.dma_start(rout.ap()[dt_i][:, c0 : c0 + cn], ro)

                # ---------- shared expert, software-pipelined ------------
                def s_hstage(t0, xt_sb):
                    gs = gsp.tile([_P, _SHT, _TC], bf16, tag="gs", name="gs_sb")
                    for hs in range(_SHT):
                        h1 = psp.tile([_P, 512], f32, tag="h1", name="h1ps")
                        for k in range(_DK):
                            nc.tensor.matmul(
                                h1,
                                sg_sb[:, k, hs * _P : (hs + 1) * _P],
                                xt_sb[:, k],
                                start=(k == 0),
                                stop=(k == _DK - 1),
                            )
                        h2 = psp.tile([_P, 512], f32, tag="h2", name="h2ps")
                        for k in range(_DK):
                            nc.tensor.matmul(
                                h2,
                                su_sb[:, k, hs * _P : (hs + 1) * _P],
                                xt_sb[:, k],
                                start=(k == 0),
                                stop=(k == _DK - 1),
                            )
                        sl = tmp.tile([_P, 512], f32, tag="sl", name="sl_sb", bufs=3)
                        nc.scalar.activation(sl, h1, Silu)
                        nc.vector.tensor_tensor(gs[:, hs], sl, h2, mult)
                    return gs

                def s_dstage(t0, gs):
                    for dt_i in range(_DK):
                        ops = psp.tile([_P, 512], f32, tag="out", name="ops", bufs=4)
                        for k in range(_SHT):
                            nc.tensor.matmul(
                                ops,
                                sd_sb[:, k, dt_i * _P : (dt_i + 1) * _P],
                                gs[:, k],
                                start=(k == 0),
                                stop=(k == _SHT - 1),
                            )
                        so = tmp.tile(
                            [_P, 512], bf16, tag="ro", name="ro_sb", bufs=6
                        )
                        if dt_i % 2:
                            nc.scalar.copy(so, ops)
                        else:
                            nc.vector.tensor_copy(so, ops)
                        nc.sync.dma_start(shout.ap()[dt_i][:, t0 : t0 + _TC], so)

                prev = None
                for ci, t0 in enumerate(range(0, _TS, _TC)):
                    gs = s_hstage(t0, xt_tiles[ci])
                    if prev is not None:
                        s_dstage(prev[0], prev[1])
                    prev = (t0, gs)
                s_dstage(prev[0], prev[1])

    orig = nc.to_json_bytes
    nc.to_json_bytes = lambda: _split_waits(orig())
    return nc


def _route(xf, w_router):
    """fp32 router matching the jax reference: softmax over logits, top-2
    (selection identical to jax.lax.top_k for non-tied logits), weights
    renormalized over the selected pair."""
    logits = xf @ w_router.T.astype(np.float32)
    m = logits.max(-1, keepdims=True)
    p = np.exp(logits - m)
    p /= p.sum(-1, keepdims=True)
    i1 = p.argmax(-1)
    p2 = p.copy()
    p2[np.arange(p.shape[0]), i1] = -1.0
    i2 = p2.argmax(-1)
    w1 = p[np.arange(p.shape[0]), i1]
    w2 = p[np.arange(p.shape[0]), i2]
    s = w1 + w2
    return i1, i2, (w1 / s).astype(np.float32), (w2 / s).astype(np.float32)


def _tile_kxm(a2d, kouter, dtype):
    """[K, M] -> [128, K//128, M] with partition dim first."""
    k, mdim = a2d.shape
    assert k == kouter * _P
    return np.ascontiguousarray(
        a2d.reshape(kouter, _P, mdim).transpose(1, 0, 2)
    ).astype(dtype)


def _pad64(n):
    return max(256, int(math.ceil(n / 64.0)) * 64)


def _prepare(inputs):
    import ml_dtypes

    bf16 = ml_dtypes.bfloat16
    f8 = ml_dtypes.float8_e4m3

    x = np.asarray(inputs["x"], dtype=np.float32)
    w_router = np.asarray(inputs["w_router"], dtype=np.float32)
    Wg = np.asarray(inputs["Wg"], dtype=np.float32)
    Wu = np.asarray(inputs["Wu"], dtype=np.float32)
    Wd = np.asarray(inputs["Wd"], dtype=np.float32)
    sg = np.asarray(inputs["sg"], dtype=np.float32)
    su = np.asarray(inputs["su"], dtype=np.float32)
    sd = np.asarray(inputs["sd"], dtype=np.float32)

    xf = np.ascontiguousarray(x.reshape(_T, _D))
    i1, i2, w1, w2 = _route(xf, w_router)

    # Segment split per expert: A = rank-1 plus high-weight rank-2 (bf16),
    # B = low-weight rank-2 (fp8).
    idxA, cwA, idxB, cwB = [], [], [], []
    for e in range(_E):
        sel1 = i1 == e
        sel2 = i2 == e
        lo = sel2 & (w2 < _THETA)
        hi = sel1 | (sel2 & ~lo)
        ia = np.nonzero(hi)[0]
        wa = np.where(sel1[ia], w1[ia], w2[ia]).astype(np.float32)
        ib = np.nonzero(lo)[0]
        wb = w2[ib].astype(np.float32)
        idxA.append(ia)
        cwA.append(wa)
        idxB.append(ib)
        cwB.append(wb)
    C1 = _pad64(max(len(i) for i in idxA))
    C2 = max(384, _pad64(max(len(i) for i in idxB)))
    C = C1 + C2

    xt_f = np.ascontiguousarray(xf.T)  # [D, T]

    in_maps = []
    for e in range(_E):
        ia, wa = idxA[e], cwA[e]
        ib, wb = idxB[e], cwB[e]
        na, nb = len(ia), len(ib)

        xe1_h = np.zeros((_P, _DK, C1), bf16)
        if na:
            xe1_h[:, :, :na] = _tile_kxm(np.ascontiguousarray(xf[ia].T), _DK, bf16)

        # fp8 segment: per-tensor scales for x and the two up-projections.
        xb = xf[ib] if nb else np.zeros((1, _D), np.float32)
        sx = 240.0 / max(np.abs(xb).max(), 1e-30)
        swg = 240.0 / max(np.abs(Wg[e]).max(), 1e-30)
        swu = 240.0 / max(np.abs(Wu[e]).max(), 1e-30)
        xe2_h = np.zeros((_P, _DK, C2), f8)
        if nb:
            xq = np.clip(xf[ib] * sx, -240, 240)
            xe2_h[:, :, :nb] = _tile_kxm(np.ascontiguousarray(xq.T), _DK, f8)

        cw_h = np.zeros((_P, C), np.float32)
        cw_h[:, :na] = wa[None, :]
        cw_h[:, C1 : C1 + nb] = (wb / (sx * swu))[None, :]
        scal_h = np.full((_P, 1), 1.0 / (sx * swg), np.float32)

        wgT = np.ascontiguousarray(Wg[e].T)  # [D, H]
        wg_h = np.ascontiguousarray(
            wgT.reshape(_DK, _P, _HT, _P).transpose(2, 1, 0, 3)
        ).astype(bf16)
        wuT = np.ascontiguousarray(Wu[e].T)
        wu_h = np.ascontiguousarray(
            wuT.reshape(_DK, _P, _HT, _P).transpose(2, 1, 0, 3)
        ).astype(bf16)
        wg8_h = np.ascontiguousarray(
            np.clip(wgT * swg, -240, 240)
            .reshape(_DK, _P, _HT, _P)
            .transpose(2, 1, 0, 3)
        ).astype(f8)
        wu8_h = np.ascontiguousarray(
            np.clip(wuT * swu, -240, 240)
            .reshape(_DK, _P, _HT, _P)
            .transpose(2, 1, 0, 3)
        ).astype(f8)
        wdT = np.ascontiguousarray(Wd[e].T)  # [H, D]
        wd_h = np.ascontiguousarray(
            wdT.reshape(_HT, _P, _DK, _P).transpose(2, 1, 0, 3)
        ).astype(bf16)

        # shared expert shard: token slice e%4, H-half e//4
        tsl = slice((e % 4) * _TS, (e % 4 + 1) * _TS)
        hsl = slice((e // 4) * _HH, (e // 4 + 1) * _HH)
        xt_h = _tile_kxm(np.ascontiguousarray(xt_f[:, tsl]), _DK, bf16)
        sg_h = _tile_kxm(np.ascontiguousarray(sg[hsl].T), _DK, bf16)
        su_h = _tile_kxm(np.ascontiguousarray(su[hsl].T), _DK, bf16)
        sd_h = _tile_kxm(np.ascontiguousarray(sd[:, hsl].T), _SHT, bf16)

        in_maps.append(
            {
                "xe1": xe1_h,
                "xe2": xe2_h,
                "cw": cw_h,
                "scal": scal_h,
                "wg1": wg_h,
                "wu1": wu_h,
                "wg8": wg8_h,
                "wu8": wu8_h,
                "wd1": wd_h,
                "xt": xt_h,
                "sgh": sg_h,
                "suh": su_h,
                "sdh": sd_h,
            }
        )
    return in_maps, (idxA, idxB, C1, C2), (C1, C2)


def _combine(results, meta):
    idxA, idxB, C1, C2 = meta
    out = np.zeros((_D, _T), np.float32)
    for e in range(_E):
        ro = results[e]["rout"].astype(np.float32).reshape(_D, C1 + C2)
        sh = results[e]["shout"].astype(np.float32).reshape(_D, _TS)
        tsl = slice((e % 4) * _TS, (e % 4 + 1) * _TS)
        out[:, tsl] += sh
        if len(idxA[e]):
            out[:, idxA[e]] += ro[:, : len(idxA[e])]
        if len(idxB[e]):
            out[:, idxB[e]] += ro[:, C1 : C1 + len(idxB[e])]
    return np.ascontiguousarray(out.T).reshape(_B, _S, _D).astype(np.float32)


def kernel(**inputs):
    from concourse import bass_utils

    in_maps, meta, caps = _prepare(inputs)
    nc = _build(*caps)
    res = bass_utils.run_bass_kernel_spmd(nc, in_maps, core_ids=list(range(_NC)))
    return _combine(res.results, meta)


# revision 28
# speedup vs baseline: 1.1371x; 1.0764x over previous
"""Trainium2 Bass kernel for nn_MoEFFNBlock (B=2,S=2048,D=1024,H=2048,E=8,K=2).

Strategy (expert-parallel, 8 cores), v2:
  host: fp32 router (matches the jax reference selection), per expert
        split its tokens into a bf16 segment (rank-1 picks plus rank-2
        picks with combine weight >= THETA) and an fp8 segment (rank-2
        picks with weight < THETA). Normalized combine weights are
        folded into the up-projection on device.
  core e: SwiGLU FFN for expert e over both segments: stage 1+2 in
        bf16 matmuls for segment A, fp8 e4m3 DoubleRow matmuls (2x PE
        rate) for segment B; stage 3 (down-proj) in bf16 over the
        concatenated segments. Plus a (tokens/4, H/2) shard of the
        shared expert: core c handles token slice c%4 and H-half c//4.
  host: scatter-add segment outputs, sum the two H-half partials of
        the shared expert.

All matmul operands are pre-tiled on host so every DMA is a large
contiguous transfer. Chunk widths are kept >= 256 columns so the
per-matmul LDWEIGHTS (97ns bf16 / 134ns fp8-DoubleRow) stays hidden
under the matmul stream.
"""

import json
import math

import numpy as np

_B, _S, _D, _H, _E = 2, 2048, 1024, 2048, 8
_T = _B * _S
_P = 128
_NC = 8
_DK = _D // _P  # 8 contraction tiles over D
_HT = _H // _P  # 16 tiles over H
_HH = _H // 2  # shared-expert H columns per core (h-half)
_SHT = _HH // _P  # 8 h-tiles per core for the shared expert
_TS = _T // 4  # shared-expert token slice per core (1024)
_TC = 512  # shared-expert token chunk
_THETA = 0.45  # rank-2 combine-weight threshold for the fp8 segment

_TPB_ENGINES = {"PE", "Activation", "DVE", "Pool", "SP"}


def _split_waits(bir_bytes: bytes) -> bytes:
    """walrus in this container accepts only one sync-wait per TPB
    instruction; Tile's tail drain carries several. Hoist extras onto
    NoOps that run just before the instruction on the same engine."""
    m = json.loads(bir_bytes)
    ctr = 0
    for f in m["functions"]:
        blocks = f["blocks"]
        items = blocks.items() if isinstance(blocks, dict) else enumerate(blocks)
        for _bname, bb in items:
            new_insts = []
            for inst in bb["instructions"]:
                si = inst.get("sync_info") or {}
                ow = si.get("on_wait") or []
                if len(ow) > 1 and inst.get("engine") in _TPB_ENGINES:
                    for w in ow[:-1]:
                        ctr += 1
                        nop = {
                            "name": f"I-waitsplit-{ctr}",
                            "engine": inst["engine"],
                            "opcode": "NoOp",
                            "ins": [],
                            "outs": [],
                            "sync_info": {"on_wait": [w], "on_update": []},
                        }
                        if "debug" in inst:
                            nop["debug"] = inst["debug"]
                        new_insts.append(nop)
                    si["on_wait"] = [ow[-1]]
                new_insts.append(inst)
            bb["instructions"] = new_insts
    return json.dumps(m).encode()


def _chunks(C):
    """Column chunks, each 256..512 wide. C must be a multiple of 64,
    C >= 256."""
    assert C >= 256 and C % 64 == 0
    out = []
    rem = C
    while rem > 512:
        w = 512 if rem >= 768 else rem - 256
        out.append(w)
        rem -= w
    out.append(rem)
    ccs, o = [], 0
    for w in out:
        assert 256 <= w <= 512
        ccs.append((o, w))
        o += w
    assert o == C
    return ccs


def _build(C1, C2):
    import concourse.bass as bass
    import concourse.mybir as mybir
    import concourse.tile as tile

    f32 = mybir.dt.float32
    bf16 = mybir.dt.bfloat16
    f8e4 = mybir.dt.float8e4
    Silu = mybir.ActivationFunctionType.Silu
    Copy = mybir.ActivationFunctionType.Copy
    mult = mybir.AluOpType.mult
    DR = mybir.MatmulPerfMode.DoubleRow

    C = C1 + C2

    nc = bass.Bass(trn_type="TRN2")
    xe1 = nc.dram_tensor("xe1", [_P, _DK, C1], bf16, kind="ExternalInput")
    xe2 = nc.dram_tensor("xe2", [_P, _DK, C2], f8e4, kind="ExternalInput")
    cw = nc.dram_tensor("cw", [_P, C], f32, kind="ExternalInput")
    scal = nc.dram_tensor("scal", [_P, 2], f32, kind="ExternalInput")
    wg1 = nc.dram_tensor("wg1", [_HT, _P, _DK, _P], bf16, kind="ExternalInput")
    wu1 = nc.dram_tensor("wu1", [_HT, _P, _DK, _P], bf16, kind="ExternalInput")
    wg8 = nc.dram_tensor("wg8", [_HT, _P, _DK, _P], f8e4, kind="ExternalInput")
    wu8 = nc.dram_tensor("wu8", [_HT, _P, _DK, _P], f8e4, kind="ExternalInput")
    wd1 = nc.dram_tensor("wd1", [_DK, _P, _HT, _P], bf16, kind="ExternalInput")
    wd8 = nc.dram_tensor("wd8", [_DK, _P, _HT, _P], f8e4, kind="ExternalInput")
    xt = nc.dram_tensor("xt", [_P, _DK, _TS], bf16, kind="ExternalInput")
    sgh = nc.dram_tensor("sgh", [_P, _DK, _HH], bf16, kind="ExternalInput")
    suh = nc.dram_tensor("suh", [_P, _DK, _HH], bf16, kind="ExternalInput")
    sdh = nc.dram_tensor("sdh", [_P, _SHT, _D], bf16, kind="ExternalInput")
    rout = nc.dram_tensor("rout", [_DK, _P, C], bf16, kind="ExternalOutput")
    shout = nc.dram_tensor("shout", [_DK, _P, _TS], bf16, kind="ExternalOutput")

    ccs1 = _chunks(C1)

    with tile.TileContext(nc) as tc:
        with (
            tc.tile_pool(name="tmp", bufs=2) as tmp,
            tc.tile_pool(name="ps", bufs=2, space="PSUM") as psp,
            tc.tile_pool(name="bigS", bufs=1) as bigS,
            tc.tile_pool(name="cwg", bufs=1) as cwg,
            tc.tile_pool(name="strDW", bufs=3) as strDW,
        ):
            # PE warmup: dummy matmuls so the PE p-state ramps while the
            # initial DMAs are in flight.
            wtile = cwg.tile([_P, 512], bf16, name="wtile")
            nc.vector.memset(wtile[:], 0.0)
            wps = psp.tile([_P, 512], f32, tag="out", name="ops", bufs=4)
            for i in range(8):
                nc.tensor.matmul(
                    wps[:],
                    wtile[:, :_P],
                    wtile[:],
                    start=(i == 0),
                    stop=(i == 7),
                )

            cw_sb = cwg.tile([_P, C], f32, name="cw_sb")
            scal_sb = cwg.tile([_P, 2], f32, name="scal_sb")
            g_sb = cwg.tile([_P, _HT, C1], bf16, name="g_sb")
            g8_sb = cwg.tile([_P, _HT, C2], f8e4, name="g8_sb")
            sg_sb = bigS.tile([_P, _DK, _HH], bf16, name="sg_sb")
            su_sb = bigS.tile([_P, _DK, _HH], bf16, name="su_sb")
            sd_sb = bigS.tile([_P, _SHT, _D], bf16, name="sd_sb")

            # ---------- segment A (bf16) stage 1 -------------------------
            with (
                tc.tile_pool(name="poolXE", bufs=1) as poolXE,
                tc.tile_pool(name="strGU", bufs=3) as strGU,
                tc.tile_pool(name="strGU8", bufs=3) as strGU8,
            ):
                # First-needed data first: xe1 chunk 0 and the first weight
                # tiles, then the rest.
                xe1_sb = poolXE.tile([_P, _DK, C1], bf16, name="xe1_sb")
                c0_, cn_ = ccs1[0]
                nc.sync.dma_start(
                    xe1_sb[:, :, c0_ : c0_ + cn_], xe1.ap()[:, :, c0_ : c0_ + cn_]
                )
                wgu_tiles = []
                for ht in range(2):
                    wg_t = strGU.tile([_P, _DK, _P], bf16, tag="wg", name="wg_t")
                    nc.sync.dma_start(wg_t[:], wg1.ap()[ht])
                    wu_t = strGU.tile([_P, _DK, _P], bf16, tag="wu", name="wu_t")
                    nc.sync.dma_start(wu_t[:], wu1.ap()[ht])
                    wgu_tiles.append((wg_t, wu_t))
                for c0_, cn_ in ccs1[1:]:
                    nc.sync.dma_start(
                        xe1_sb[:, :, c0_ : c0_ + cn_], xe1.ap()[:, :, c0_ : c0_ + cn_]
                    )
                nc.sync.dma_start(cw_sb[:], cw.ap())
                nc.sync.dma_start(scal_sb[:], scal.ap())
                xe2_sb = poolXE.tile([_P, _DK, C2], f8e4, name="xe2_sb")
                nc.sync.dma_start(xe2_sb[:], xe2.ap())

                for ht in range(_HT):
                    if ht < 2:
                        wg_t, wu_t = wgu_tiles[ht]
                    else:
                        wg_t = strGU.tile([_P, _DK, _P], bf16, tag="wg", name="wg_t")
                        nc.sync.dma_start(wg_t[:], wg1.ap()[ht])
                        wu_t = strGU.tile([_P, _DK, _P], bf16, tag="wu", name="wu_t")
                        nc.sync.dma_start(wu_t[:], wu1.ap()[ht])
                    for c0, cn in ccs1:
                        h1 = psp.tile([_P, 512], f32, tag="h1", name="h1ps")[:, :cn]
                        for k in range(_DK):
                            nc.tensor.matmul(
                                h1,
                                wg_t[:, k],
                                xe1_sb[:, k, c0 : c0 + cn],
                                start=(k == 0),
                                stop=(k == _DK - 1),
                            )
                        h2 = psp.tile([_P, 512], f32, tag="h2", name="h2ps")[:, :cn]
                        for k in range(_DK):
                            nc.tensor.matmul(
                                h2,
                                wu_t[:, k],
                                xe1_sb[:, k, c0 : c0 + cn],
                                start=(k == 0),
                                stop=(k == _DK - 1),
                            )
                        sl = tmp.tile([_P, 512], f32, tag="sl", name="sl_sb", bufs=3)[
                            :, :cn
                        ]
                        nc.scalar.activation(sl, h1, Silu)
                        t2 = tmp.tile([_P, 512], f32, tag="t2", name="t2_sb", bufs=3)[
                            :, :cn
                        ]
                        nc.vector.tensor_tensor(t2, h2, cw_sb[:, c0 : c0 + cn], mult)
                        nc.vector.tensor_tensor(
                            g_sb[:, ht, c0 : c0 + cn], sl, t2, mult
                        )

                # ---------- segment B (fp8 DoubleRow) stage 1 ------------
                for ht in range(_HT):
                    wg8_t = strGU8.tile([_P, _DK, _P], f8e4, tag="wg8", name="wg8_t")
                    nc.sync.dma_start(wg8_t[:], wg8.ap()[ht])
                    wu8_t = strGU8.tile([_P, _DK, _P], f8e4, tag="wu8", name="wu8_t")
                    nc.sync.dma_start(wu8_t[:], wu8.ap()[ht])
                    h1 = psp.tile([_P, 512], f32, tag="h1", name="h1ps")[:, :C2]
                    for k in range(_DK // 2):
                        nc.tensor.matmul(
                            h1,
                            wg8_t[:, 2 * k : 2 * k + 2],
                            xe2_sb[:, 2 * k : 2 * k + 2],
                            start=(k == 0),
                            stop=(k == _DK // 2 - 1),
                            perf_mode=DR,
                        )
                    h2 = psp.tile([_P, 512], f32, tag="h2", name="h2ps")[:, :C2]
                    for k in range(_DK // 2):
                        nc.tensor.matmul(
                            h2,
                            wu8_t[:, 2 * k : 2 * k + 2],
                            xe2_sb[:, 2 * k : 2 * k + 2],
                            start=(k == 0),
                            stop=(k == _DK // 2 - 1),
                            perf_mode=DR,
                        )
                    sl = tmp.tile([_P, 512], f32, tag="sl", name="sl_sb", bufs=3)[
                        :, :C2
                    ]
                    nc.scalar.activation(sl, h1, Silu, scale=scal_sb[:, 0:1])
                    t2 = tmp.tile([_P, 512], f32, tag="t2", name="t2_sb", bufs=3)[
                        :, :C2
                    ]
                    nc.vector.tensor_tensor(t2, h2, cw_sb[:, C1:C], mult)
                    nc.vector.tensor_tensor(g8_sb[:, ht], sl, t2, mult)

            # ---------- stage 2 (down-proj, both segments) ---------------
            with (
                tc.tile_pool(name="strS", bufs=2) as strS,
                tc.tile_pool(name="gsp", bufs=2) as gsp,
            ):
                xt_tiles = [
                    strS.tile([_P, _DK, _TC], bf16, tag="xt", name="xt_sb")
                    for _ in range(0, _TS, _TC)
                ]

                for dt_i in range(_DK):
                    wd_t = strDW.tile([_P, _HT, _P], bf16, tag="wd", name="wd_t")
                    nc.sync.dma_start(wd_t[:], wd1.ap()[dt_i])
                    wd8_t = strDW.tile([_P, _HT, _P], f8e4, tag="wd8", name="wd8_t")
                    nc.sync.dma_start(wd8_t[:], wd8.ap()[dt_i])
                    # Interleave the bulk shared-expert loads between the
                    # down-proj weight tiles so neither starves the other.
                    if dt_i == 0:
                        nc.sync.dma_start(sg_sb[:], sgh.ap())
                    elif dt_i == 2:
                        nc.sync.dma_start(su_sb[:], suh.ap())
                    elif dt_i == 4:
                        nc.sync.dma_start(sd_sb[:], sdh.ap())
                    elif dt_i == 5:
                        nc.sync.dma_start(xt_tiles[0][:], xt.ap()[:, :, 0:_TC])
                    elif dt_i == 6:
                        nc.sync.dma_start(xt_tiles[1][:], xt.ap()[:, :, _TC : 2 * _TC])
                    for c0, cn in ccs1:
                        ops = psp.tile(
                            [_P, 512], f32, tag="out", name="ops", bufs=4
                        )[:, :cn]
                        for k in range(_HT):
                            nc.tensor.matmul(
                                ops,
                                wd_t[:, k],
                                g_sb[:, k, c0 : c0 + cn],
                                start=(k == 0),
                                stop=(k == _HT - 1),
                            )
                        ro = tmp.tile(
                            [_P, 512], bf16, tag="ro", name="ro_sb", bufs=6
                        )[:, :cn]
                        nc.vector.tensor_copy(ro, ops)
                        nc.sync.dma_start(rout.ap()[dt_i][:, c0 : c0 + cn], ro)
                    # fp8 segment down-proj: DoubleRow over the 16 h-tiles,
                    # descaled on the Activation engine during the copy.
                    ops = psp.tile([_P, 512], f32, tag="out", name="ops", bufs=4)[
                        :, :C2
                    ]
                    for k in range(_HT // 2):
                        nc.tensor.matmul(
                            ops,
                            wd8_t[:, 2 * k : 2 * k + 2],
                            g8_sb[:, 2 * k : 2 * k + 2],
                            start=(k == 0),
                            stop=(k == _HT // 2 - 1),
                            perf_mode=DR,
                        )
                    ro = tmp.tile([_P, 512], bf16, tag="ro", name="ro_sb", bufs=6)[
                        :, :C2
                    ]
                    nc.scalar.activation(ro, ops, Copy, scale=scal_sb[:, 1:2])
                    nc.sync.dma_start(rout.ap()[dt_i][:, C1:C], ro)

                # ---------- shared expert, software-pipelined ------------
                def s_hstage(t0, xt_sb):
                    gs = gsp.tile([_P, _SHT, _TC], bf16, tag="gs", name="gs_sb")
                    for hs in range(_SHT):
                        h1 = psp.tile([_P, 512], f32, tag="h1", name="h1ps")
                        for k in range(_DK):
                            nc.tensor.matmul(
                                h1,
                                sg_sb[:, k, hs * _P : (hs + 1) * _P],
                                xt_sb[:, k],
                                start=(k == 0),
                                stop=(k == _DK - 1),
                            )
                        h2 = psp.tile([_P, 512], f32, tag="h2", name="h2ps")
                        for k in range(_DK):
                            nc.tensor.matmul(
                                h2,
                                su_sb[:, k, hs * _P : (hs + 1) * _P],
                                xt_sb[:, k],
                                start=(k == 0),
                                stop=(k == _DK - 1),
                            )
                        sl = tmp.tile([_P, 512], f32, tag="sl", name="sl_sb", bufs=3)
                        nc.scalar.activation(sl, h1, Silu)
                        nc.vector.tensor_tensor(gs[:, hs], sl, h2, mult)
                    return gs

                def s_dstage(t0, gs):
                    for dt_i in range(_DK):
                        ops = psp.tile([_P, 512], f32, tag="out", name="ops", bufs=4)
                        for k in range(_SHT):
                            nc.tensor.matmul(
                                ops,
                                sd_sb[:, k, dt_i * _P : (dt_i + 1) * _P],
                                gs[:, k],
                                start=(k == 0),
                                stop=(k == _SHT - 1),
                            )
                        so = tmp.tile(
                            [_P, 512], bf16, tag="ro", name="ro_sb", bufs=6
                        )
                        if dt_i % 2:
                            nc.scalar.copy(so, ops)
                        else:
                            nc.vector.tensor_copy(so, ops)
                        nc.sync.dma_start(shout.ap()[dt_i][:, t0 : t0 + _TC], so)

                prev = None
                for ci, t0 in enumerate(range(0, _TS, _TC)):
                    gs = s_hstage(t0, xt_tiles[ci])
                    if prev is not None:
                        s_dstage(prev[0], prev[1])
                    prev = (t0, gs)
                s_dstage(prev[0], prev[1])

    orig = nc.to_json_bytes
    nc.to_json_bytes = lambda: _split_waits(orig())
    return nc


def _route(xf, w_router):
    """fp32 router matching the jax reference: softmax over logits, top-2
    (selection identical to jax.lax.top_k for non-tied logits), weights
    renormalized over the selected pair."""
    logits = xf @ w_router.T.astype(np.float32)
    m = logits.max(-1, keepdims=True)
    p = np.exp(logits - m)
    p /= p.sum(-1, keepdims=True)
    i1 = p.argmax(-1)
    p2 = p.copy()
    p2[np.arange(p.shape[0]), i1] = -1.0
    i2 = p2.argmax(-1)
    w1 = p[np.arange(p.shape[0]), i1]
    w2 = p[np.arange(p.shape[0]), i2]
    s = w1 + w2
    return i1, i2, (w1 / s).astype(np.float32), (w2 / s).astype(np.float32)


def _tile_kxm(a2d, kouter, dtype):
    """[K, M] -> [128, K//128, M] with partition dim first."""
    k, mdim = a2d.shape
    assert k == kouter * _P
    return np.ascontiguousarray(
        a2d.reshape(kouter, _P, mdim).transpose(1, 0, 2)
    ).astype(dtype)


def _pad64(n):
    return max(256, int(math.ceil(n / 64.0)) * 64)


def _prepare(inputs):
    import ml_dtypes

    bf16 = ml_dtypes.bfloat16
    f8 = ml_dtypes.float8_e4m3

    x = np.asarray(inputs["x"], dtype=np.float32)
    w_router = np.asarray(inputs["w_router"], dtype=np.float32)
    Wg = np.asarray(inputs["Wg"], dtype=np.float32)
    Wu = np.asarray(inputs["Wu"], dtype=np.float32)
    Wd = np.asarray(inputs["Wd"], dtype=np.float32)
    sg = np.asarray(inputs["sg"], dtype=np.float32)
    su = np.asarray(inputs["su"], dtype=np.float32)
    sd = np.asarray(inputs["sd"], dtype=np.float32)

    xf = np.ascontiguousarray(x.reshape(_T, _D))
    i1, i2, w1, w2 = _route(xf, w_router)

    # Segment split per expert: A (bf16) = all rank-1 picks plus the
    # highest-weight rank-2 picks, filled to a common capacity so every
    # core does identical work; B (fp8) = the remaining low-weight
    # rank-2 picks.
    l1max = max(int((i1 == e).sum()) for e in range(_E))
    C1 = max(_pad64(l1max), 640)
    idxA, cwA, idxB, cwB = [], [], [], []
    for e in range(_E):
        ia1 = np.nonzero(i1 == e)[0]
        ia2 = np.nonzero(i2 == e)[0]
        order = ia2[np.argsort(-w2[ia2], kind="stable")]
        nfill = max(0, C1 - len(ia1))
        ia = np.concatenate([ia1, order[:nfill]])
        ib = np.sort(order[nfill:])
        wa = np.where(i1[ia] == e, w1[ia], w2[ia]).astype(np.float32)
        wb = w2[ib].astype(np.float32)
        idxA.append(ia)
        cwA.append(wa)
        idxB.append(ib)
        cwB.append(wb)
    C2 = max(384, _pad64(max(len(i) for i in idxB)))
    C = C1 + C2

    xt_f = np.ascontiguousarray(xf.T)  # [D, T]

    in_maps = []
    for e in range(_E):
        ia, wa = idxA[e], cwA[e]
        ib, wb = idxB[e], cwB[e]
        na, nb = len(ia), len(ib)

        xe1_h = np.zeros((_P, _DK, C1), bf16)
        if na:
            xe1_h[:, :, :na] = _tile_kxm(np.ascontiguousarray(xf[ia].T), _DK, bf16)

        # fp8 segment: per-tensor scales for x and the three projections.
        xb = xf[ib] if nb else np.zeros((1, _D), np.float32)
        sx = 240.0 / max(np.abs(xb).max(), 1e-30)
        swg = 240.0 / max(np.abs(Wg[e]).max(), 1e-30)
        swu = 240.0 / max(np.abs(Wu[e]).max(), 1e-30)
        swd = 240.0 / max(np.abs(Wd[e]).max(), 1e-30)
        xe2_h = np.zeros((_P, _DK, C2), f8)
        if nb:
            xq = np.clip(xf[ib] * sx, -240, 240)
            xe2_h[:, :, :nb] = _tile_kxm(np.ascontiguousarray(xq.T), _DK, f8)

        # Exact absmax of the fp8 segment's gated activation so the
        # on-device fp8 quantize of g (folded into cw) cannot overflow.
        if nb:
            gg = xb @ Wg[e].T
            uu = xb @ Wu[e].T
            hseg = (gg / (1.0 + np.exp(-gg))) * uu * wb[:, None]
            am = max(float(np.abs(hseg).max()), 1e-30)
        else:
            am = 1.0
        s_g = 240.0 / (1.3 * am)

        cw_h = np.zeros((_P, C), np.float32)
        cw_h[:, :na] = wa[None, :]
        cw_h[:, C1 : C1 + nb] = (wb * s_g / (sx * swu))[None, :]
        scal_h = np.tile(
            np.array([1.0 / (sx * swg), 1.0 / (s_g * swd)], np.float32), (_P, 1)
        )

        wgT = np.ascontiguousarray(Wg[e].T)  # [D, H]
        wg_h = np.ascontiguousarray(
            wgT.reshape(_DK, _P, _HT, _P).transpose(2, 1, 0, 3)
        ).astype(bf16)
        wuT = np.ascontiguousarray(Wu[e].T)
        wu_h = np.ascontiguousarray(
            wuT.reshape(_DK, _P, _HT, _P).transpose(2, 1, 0, 3)
        ).astype(bf16)
        wg8_h = np.ascontiguousarray(
            np.clip(wgT * swg, -240, 240)
            .reshape(_DK, _P, _HT, _P)
            .transpose(2, 1, 0, 3)
        ).astype(f8)
        wu8_h = np.ascontiguousarray(
            np.clip(wuT * swu, -240, 240)
            .reshape(_DK, _P, _HT, _P)
            .transpose(2, 1, 0, 3)
        ).astype(f8)
        wdT = np.ascontiguousarray(Wd[e].T)  # [H, D]
        wd_h = np.ascontiguousarray(
            wdT.reshape(_HT, _P, _DK, _P).transpose(2, 1, 0, 3)
        ).astype(bf16)
        wd8_h = np.ascontiguousarray(
            np.clip(wdT * swd, -240, 240)
            .reshape(_HT, _P, _DK, _P)
            .transpose(2, 1, 0, 3)
        ).astype(f8)

        # shared expert shard: token slice e%4, H-half e//4
        tsl = slice((e % 4) * _TS, (e % 4 + 1) * _TS)
        hsl = slice((e // 4) * _HH, (e // 4 + 1) * _HH)
        xt_h = _tile_kxm(np.ascontiguousarray(xt_f[:, tsl]), _DK, bf16)
        sg_h = _tile_kxm(np.ascontiguousarray(sg[hsl].T), _DK, bf16)
        su_h = _tile_kxm(np.ascontiguousarray(su[hsl].T), _DK, bf16)
        sd_h = _tile_kxm(np.ascontiguousarray(sd[:, hsl].T), _SHT, bf16)

        in_maps.append(
            {
                "xe1": xe1_h,
                "xe2": xe2_h,
                "cw": cw_h,
                "scal": scal_h,
                "wg1": wg_h,
                "wu1": wu_h,
                "wg8": wg8_h,
                "wu8": wu8_h,
                "wd1": wd_h,
                "wd8": wd8_h,
                "xt": xt_h,
                "sgh": sg_h,
                "suh": su_h,
                "sdh": sd_h,
            }
        )
    return in_maps, (idxA, idxB, C1, C2), (C1, C2)


def _combine(results, meta):
    idxA, idxB, C1, C2 = meta
    out = np.zeros((_D, _T), np.float32)
    for e in range(_E):
        ro = results[e]["rout"].astype(np.float32).reshape(_D, C1 + C2)
        sh = results[e]["shout"].astype(np.float32).reshape(_D, _TS)
        tsl = slice((e % 4) * _TS, (e % 4 + 1) * _TS)
        out[:, tsl] += sh
        if len(idxA[e]):
            out[:, idxA[e]] += ro[:, : len(idxA[e])]
        if len(idxB[e]):
            out[:, idxB[e]] += ro[:, C1 : C1 + len(idxB[e])]
    return np.ascontiguousarray(out.T).reshape(_B, _S, _D).astype(np.float32)


def kernel(**inputs):
    from concourse import bass_utils

    in_maps, meta, caps = _prepare(inputs)
    nc = _build(*caps)
    res = bass_utils.run_bass_kernel_spmd(nc, in_maps, core_ids=list(range(_NC)))
    return _combine(res.results, meta)


# revision 33
# speedup vs baseline: 1.1602x; 1.0203x over previous
"""Trainium2 Bass kernel for nn_MoEFFNBlock (B=2,S=2048,D=1024,H=2048,E=8,K=2).

Strategy (expert-parallel, 8 cores):
  host: fp32 router (matches the jax reference selection). Per expert,
        tokens split into segment A (bf16): all rank-1 picks plus the
        highest-combine-weight rank-2 picks, filled to a common
        capacity C1 so every core does identical work; and segment B
        (fp8 e4m3): the remaining low-weight rank-2 picks (weight
        <= ~0.45, so fp8's ~4% path error contributes ~1.3% to the
        output norm). Normalized combine weights and all fp8 descales
        are folded into the device-side up-projection multiply.
  core e: SwiGLU FFN for expert e: segment A in bf16 matmuls, segment
        B entirely in fp8 DoubleRow matmuls (2x PE rate; g quantized
        to fp8 on device with a host-verified scale). Plus a
        (tokens/4, H/2) shard of the shared expert: core c handles
        token slice c%4 and H-half c//4, all bf16.
  host: scatter-add segment outputs, sum the two H-half partials of
        the shared expert.

All matmul operands are pre-tiled on host so every DMA is a large
contiguous transfer, issued on one FIFO ring in consumption order with
bulk loads interleaved between weight tiles. Chunk widths stay >= 256
columns so the per-matmul LDWEIGHTS (97ns bf16 / 134ns fp8-DoubleRow)
hides under the matmul stream.
"""

import json
import math

import numpy as np

_B, _S, _D, _H, _E = 2, 2048, 1024, 2048, 8
_T = _B * _S
_P = 128
_NC = 8
_DK = _D // _P  # 8 contraction tiles over D
_HT = _H // _P  # 16 tiles over H
_HH = _H // 2  # shared-expert H columns per core (h-half)
_SHT = _HH // _P  # 8 h-tiles per core for the shared expert
_TS = _T // 4  # shared-expert token slice per core (1024)
_TC = 512  # shared-expert token chunk

_TPB_ENGINES = {"PE", "Activation", "DVE", "Pool", "SP"}


def _split_waits(bir_bytes: bytes) -> bytes:
    """walrus in this container accepts only one sync-wait per TPB
    instruction; Tile's tail drain carries several. Hoist extras onto
    NoOps that run just before the instruction on the same engine."""
    m = json.loads(bir_bytes)
    ctr = 0
    for f in m["functions"]:
        blocks = f["blocks"]
        items = blocks.items() if isinstance(blocks, dict) else enumerate(blocks)
        for _bname, bb in items:
            new_insts = []
            for inst in bb["instructions"]:
                si = inst.get("sync_info") or {}
                ow = si.get("on_wait") or []
                if len(ow) > 1 and inst.get("engine") in _TPB_ENGINES:
                    for w in ow[:-1]:
                        ctr += 1
                        nop = {
                            "name": f"I-waitsplit-{ctr}",
                            "engine": inst["engine"],
                            "opcode": "NoOp",
                            "ins": [],
                            "outs": [],
                            "sync_info": {"on_wait": [w], "on_update": []},
                        }
                        if "debug" in inst:
                            nop["debug"] = inst["debug"]
                        new_insts.append(nop)
                    si["on_wait"] = [ow[-1]]
                new_insts.append(inst)
            bb["instructions"] = new_insts
    return json.dumps(m).encode()


def _chunks(C):
    """Column chunks, each 256..512 wide, smallest first (so the first
    chunk's DMA lands soonest at startup). C must be a multiple of 64,
    C >= 256."""
    assert C >= 256 and C % 64 == 0
    out = []
    rem = C
    while rem > 512:
        w = 512 if rem >= 768 else rem - 256
        out.append(w)
        rem -= w
    out.append(rem)
    ccs, o = [], 0
    for w in sorted(out):
        assert 256 <= w <= 512
        ccs.append((o, w))
        o += w
    assert o == C
    return ccs


def _build(C1, C2):
    import concourse.bass as bass
    import concourse.mybir as mybir
    import concourse.tile as tile

    f32 = mybir.dt.float32
    bf16 = mybir.dt.bfloat16
    f8e4 = mybir.dt.float8e4
    Silu = mybir.ActivationFunctionType.Silu
    Copy = mybir.ActivationFunctionType.Copy
    mult = mybir.AluOpType.mult
    DR = mybir.MatmulPerfMode.DoubleRow

    C = C1 + C2

    nc = bass.Bass(trn_type="TRN2")
    xe1 = nc.dram_tensor("xe1", [_P, _DK, C1], bf16, kind="ExternalInput")
    xe2 = nc.dram_tensor("xe2", [_P, _DK, C2], f8e4, kind="ExternalInput")
    cw = nc.dram_tensor("cw", [_P, C], f32, kind="ExternalInput")
    scal = nc.dram_tensor("scal", [_P, 2], f32, kind="ExternalInput")
    wg1 = nc.dram_tensor("wg1", [_HT, _P, _DK, _P], bf16, kind="ExternalInput")
    wu1 = nc.dram_tensor("wu1", [_HT, _P, _DK, _P], bf16, kind="ExternalInput")
    wg8 = nc.dram_tensor("wg8", [_HT, _P, _DK, _P], f8e4, kind="ExternalInput")
    wu8 = nc.dram_tensor("wu8", [_HT, _P, _DK, _P], f8e4, kind="ExternalInput")
    wd1 = nc.dram_tensor("wd1", [_DK, _P, _HT, _P], bf16, kind="ExternalInput")
    wd8 = nc.dram_tensor("wd8", [_DK, _P, _HT, _P], f8e4, kind="ExternalInput")
    xt = nc.dram_tensor("xt", [_P, _DK, _TS], bf16, kind="ExternalInput")
    sgh = nc.dram_tensor("sgh", [_P, _DK, _HH], bf16, kind="ExternalInput")
    suh = nc.dram_tensor("suh", [_P, _DK, _HH], bf16, kind="ExternalInput")
    sdh = nc.dram_tensor("sdh", [_P, _SHT, _D], bf16, kind="ExternalInput")
    rout = nc.dram_tensor("rout", [_DK, _P, C], bf16, kind="ExternalOutput")
    shout = nc.dram_tensor("shout", [_DK, _P, _TS], bf16, kind="ExternalOutput")

    ccs1 = _chunks(C1)

    with tile.TileContext(nc) as tc:
        with (
            tc.tile_pool(name="tmp", bufs=2) as tmp,
            tc.tile_pool(name="ps", bufs=2, space="PSUM") as psp,
            tc.tile_pool(name="bigS", bufs=1) as bigS,
            tc.tile_pool(name="cwg", bufs=1) as cwg,
            tc.tile_pool(name="strDW", bufs=3) as strDW,
        ):
            # PE warmup: dummy matmuls so the PE p-state ramps while the
            # initial DMAs are in flight.
            wtile = cwg.tile([_P, 512], bf16, name="wtile")
            nc.vector.memset(wtile[:], 0.0)
            wps = psp.tile([_P, 512], f32, tag="out", name="ops", bufs=4)
            for i in range(8):
                nc.tensor.matmul(
                    wps[:],
                    wtile[:, :_P],
                    wtile[:],
                    start=(i == 0),
                    stop=(i == 7),
                )

            cw_sb = cwg.tile([_P, C], f32, name="cw_sb")
            scal_sb = cwg.tile([_P, 2], f32, name="scal_sb")
            g_sb = cwg.tile([_P, _HT, C1], bf16, name="g_sb")
            g8_sb = cwg.tile([_P, _HT, C2], f8e4, name="g8_sb")
            sg_sb = bigS.tile([_P, _DK, _HH], bf16, name="sg_sb")
            su_sb = bigS.tile([_P, _DK, _HH], bf16, name="su_sb")
            sd_sb = bigS.tile([_P, _SHT, _D], bf16, name="sd_sb")

            # ---------- segment A (bf16) stage 1 -------------------------
            with (
                tc.tile_pool(name="poolXE", bufs=1) as poolXE,
                tc.tile_pool(name="strGU", bufs=3) as strGU,
                tc.tile_pool(name="strGU8", bufs=3) as strGU8,
            ):
                # First-needed data first: xe1 chunk 0 and the first weight
                # tiles, then the rest.
                # Startup-critical order: first (smallest) xe1 chunk and the
                # ht0 weight tiles lead; the rest interleaves with the early
                # weight stream so neither starves.
                xe1_sb = poolXE.tile([_P, _DK, C1], bf16, name="xe1_sb")
                xe2_sb = poolXE.tile([_P, _DK, C2], f8e4, name="xe2_sb")
                c0_, cn_ = ccs1[0]
                nc.sync.dma_start(
                    xe1_sb[:, :, c0_ : c0_ + cn_], xe1.ap()[:, :, c0_ : c0_ + cn_]
                )
                wgu_tiles = []
                for ht in range(3):
                    wg_t = strGU.tile([_P, _DK, _P], bf16, tag="wg", name="wg_t")
                    nc.sync.dma_start(wg_t[:], wg1.ap()[ht])
                    wu_t = strGU.tile([_P, _DK, _P], bf16, tag="wu", name="wu_t")
                    nc.sync.dma_start(wu_t[:], wu1.ap()[ht])
                    wgu_tiles.append((wg_t, wu_t))
                    if ht == 0:
                        for c0_, cn_ in ccs1[1:]:
                            nc.sync.dma_start(
                                xe1_sb[:, :, c0_ : c0_ + cn_],
                                xe1.ap()[:, :, c0_ : c0_ + cn_],
                            )
                    elif ht == 1:
                        nc.sync.dma_start(cw_sb[:], cw.ap())
                        nc.sync.dma_start(scal_sb[:], scal.ap())
                    elif ht == 2:
                        nc.sync.dma_start(xe2_sb[:], xe2.ap())

                for ht in range(_HT):
                    if ht < 3:
                        wg_t, wu_t = wgu_tiles[ht]
                    else:
                        wg_t = strGU.tile([_P, _DK, _P], bf16, tag="wg", name="wg_t")
                        nc.sync.dma_start(wg_t[:], wg1.ap()[ht])
                        wu_t = strGU.tile([_P, _DK, _P], bf16, tag="wu", name="wu_t")
                        nc.sync.dma_start(wu_t[:], wu1.ap()[ht])
                    for c0, cn in ccs1:
                        h1 = psp.tile([_P, 512], f32, tag="h1", name="h1ps")[:, :cn]
                        for k in range(_DK):
                            nc.tensor.matmul(
                                h1,
                                wg_t[:, k],
                                xe1_sb[:, k, c0 : c0 + cn],
                                start=(k == 0),
                                stop=(k == _DK - 1),
                            )
                        h2 = psp.tile([_P, 512], f32, tag="h2", name="h2ps")[:, :cn]
                        for k in range(_DK):
                            nc.tensor.matmul(
                                h2,
                                wu_t[:, k],
                                xe1_sb[:, k, c0 : c0 + cn],
                                start=(k == 0),
                                stop=(k == _DK - 1),
                            )
                        sl = tmp.tile([_P, 512], f32, tag="sl", name="sl_sb", bufs=3)[
                            :, :cn
                        ]
                        nc.scalar.activation(sl, h1, Silu)
                        t2 = tmp.tile([_P, 512], f32, tag="t2", name="t2_sb", bufs=3)[
                            :, :cn
                        ]
                        nc.vector.tensor_tensor(t2, h2, cw_sb[:, c0 : c0 + cn], mult)
                        nc.vector.tensor_tensor(
                            g_sb[:, ht, c0 : c0 + cn], sl, t2, mult
                        )

                # ---------- segment B (fp8 DoubleRow) stage 1 ------------
                for ht in range(_HT):
                    wg8_t = strGU8.tile([_P, _DK, _P], f8e4, tag="wg8", name="wg8_t")
                    nc.sync.dma_start(wg8_t[:], wg8.ap()[ht])
                    wu8_t = strGU8.tile([_P, _DK, _P], f8e4, tag="wu8", name="wu8_t")
                    nc.sync.dma_start(wu8_t[:], wu8.ap()[ht])
                    h1 = psp.tile([_P, 512], f32, tag="h1", name="h1ps")[:, :C2]
                    for k in range(_DK // 2):
                        nc.tensor.matmul(
                            h1,
                            wg8_t[:, 2 * k : 2 * k + 2],
                            xe2_sb[:, 2 * k : 2 * k + 2],
                            start=(k == 0),
                            stop=(k == _DK // 2 - 1),
                            perf_mode=DR,
                        )
                    h2 = psp.tile([_P, 512], f32, tag="h2", name="h2ps")[:, :C2]
                    for k in range(_DK // 2):
                        nc.tensor.matmul(
                            h2,
                            wu8_t[:, 2 * k : 2 * k + 2],
                            xe2_sb[:, 2 * k : 2 * k + 2],
                            start=(k == 0),
                            stop=(k == _DK // 2 - 1),
                            perf_mode=DR,
                        )
                    sl = tmp.tile([_P, 512], f32, tag="sl", name="sl_sb", bufs=3)[
                        :, :C2
                    ]
                    nc.scalar.activation(sl, h1, Silu, scale=scal_sb[:, 0:1])
                    t2 = tmp.tile([_P, 512], f32, tag="t2", name="t2_sb", bufs=3)[
                        :, :C2
                    ]
                    nc.vector.tensor_tensor(t2, h2, cw_sb[:, C1:C], mult)
                    nc.vector.tensor_tensor(g8_sb[:, ht], sl, t2, mult)

            # ---------- stage 2 (down-proj, both segments) ---------------
            with (
                tc.tile_pool(name="strS", bufs=2) as strS,
                tc.tile_pool(name="gsp", bufs=2) as gsp,
            ):
                xt_tiles = [
                    strS.tile([_P, _DK, _TC], bf16, tag="xt", name="xt_sb")
                    for _ in range(0, _TS, _TC)
                ]

                for dt_i in range(_DK):
                    wd_t = strDW.tile([_P, _HT, _P], bf16, tag="wd", name="wd_t")
                    nc.sync.dma_start(wd_t[:], wd1.ap()[dt_i])
                    wd8_t = strDW.tile([_P, _HT, _P], f8e4, tag="wd8", name="wd8_t")
                    nc.sync.dma_start(wd8_t[:], wd8.ap()[dt_i])
                    # Interleave the bulk shared-expert loads between the
                    # down-proj weight tiles so neither starves the other.
                    if dt_i == 0:
                        nc.sync.dma_start(sg_sb[:], sgh.ap())
                    elif dt_i == 2:
                        nc.sync.dma_start(su_sb[:], suh.ap())
                    elif dt_i == 4:
                        nc.sync.dma_start(sd_sb[:], sdh.ap())
                    elif dt_i == 5:
                        nc.sync.dma_start(xt_tiles[0][:], xt.ap()[:, :, 0:_TC])
                    elif dt_i == 6:
                        nc.sync.dma_start(xt_tiles[1][:], xt.ap()[:, :, _TC : 2 * _TC])
                    for c0, cn in ccs1:
                        ops = psp.tile(
                            [_P, 512], f32, tag="out", name="ops", bufs=4
                        )[:, :cn]
                        for k in range(_HT):
                            nc.tensor.matmul(
                                ops,
                                wd_t[:, k],
                                g_sb[:, k, c0 : c0 + cn],
                                start=(k == 0),
                                stop=(k == _HT - 1),
                            )
                        ro = tmp.tile(
                            [_P, 512], bf16, tag="ro", name="ro_sb", bufs=6
                        )[:, :cn]
                        nc.vector.tensor_copy(ro, ops)
                        nc.sync.dma_start(rout.ap()[dt_i][:, c0 : c0 + cn], ro)
                    # fp8 segment down-proj: DoubleRow over the 16 h-tiles,
                    # descaled on the Activation engine during the copy.
                    ops = psp.tile([_P, 512], f32, tag="out", name="ops", bufs=4)[
                        :, :C2
                    ]
                    for k in range(_HT // 2):
                        nc.tensor.matmul(
                            ops,
                            wd8_t[:, 2 * k : 2 * k + 2],
                            g8_sb[:, 2 * k : 2 * k + 2],
                            start=(k == 0),
                            stop=(k == _HT // 2 - 1),
                            perf_mode=DR,
                        )
                    ro = tmp.tile([_P, 512], bf16, tag="ro", name="ro_sb", bufs=6)[
                        :, :C2
                    ]
                    nc.scalar.activation(ro, ops, Copy, scale=scal_sb[:, 1:2])
                    nc.sync.dma_start(rout.ap()[dt_i][:, C1:C], ro)

                # ---------- shared expert, software-pipelined ------------
                def s_hstage(t0, xt_sb):
                    gs = gsp.tile([_P, _SHT, _TC], bf16, tag="gs", name="gs_sb")
                    for hs in range(_SHT):
                        h1 = psp.tile([_P, 512], f32, tag="h1", name="h1ps")
                        for k in range(_DK):
                            nc.tensor.matmul(
                                h1,
                                sg_sb[:, k, hs * _P : (hs + 1) * _P],
                                xt_sb[:, k],
                                start=(k == 0),
                                stop=(k == _DK - 1),
                            )
                        h2 = psp.tile([_P, 512], f32, tag="h2", name="h2ps")
                        for k in range(_DK):
                            nc.tensor.matmul(
                                h2,
                                su_sb[:, k, hs * _P : (hs + 1) * _P],
                                xt_sb[:, k],
                                start=(k == 0),
                                stop=(k == _DK - 1),
                            )
                        sl = tmp.tile([_P, 512], f32, tag="sl", name="sl_sb", bufs=3)
                        nc.scalar.activation(sl, h1, Silu)
                        nc.vector.tensor_tensor(gs[:, hs], sl, h2, mult)
                    return gs

                def s_dstage(t0, gs):
                    for dt_i in range(_DK):
                        ops = psp.tile([_P, 512], f32, tag="out", name="ops", bufs=4)
                        for k in range(_SHT):
                            nc.tensor.matmul(
                                ops,
                                sd_sb[:, k, dt_i * _P : (dt_i + 1) * _P],
                                gs[:, k],
                                start=(k == 0),
                                stop=(k == _SHT - 1),
                            )
                        so = tmp.tile(
                            [_P, 512], bf16, tag="ro", name="ro_sb", bufs=6
                        )
                        if dt_i % 2:
                            nc.scalar.copy(so, ops)
                        else:
                            nc.vector.tensor_copy(so, ops)
                        nc.sync.dma_start(shout.ap()[dt_i][:, t0 : t0 + _TC], so)

                prev = None
                for ci, t0 in enumerate(range(0, _TS, _TC)):
                    gs = s_hstage(t0, xt_tiles[ci])
                    if prev is not None:
                        s_dstage(prev[0], prev[1])
                    prev = (t0, gs)
                s_dstage(prev[0], prev[1])

    orig = nc.to_json_bytes
    nc.to_json_bytes = lambda: _split_waits(orig())
    return nc


def _route(xf, w_router):
    """fp32 router matching the jax reference: softmax over logits, top-2
    (selection identical to jax.lax.top_k for non-tied logits), weights
    renormalized over the selected pair."""
    logits = xf @ w_router.T.astype(np.float32)
    m = logits.max(-1, keepdims=True)
    p = np.exp(logits - m)
    p /= p.sum(-1, keepdims=True)
    i1 = p.argmax(-1)
    p2 = p.copy()
    p2[np.arange(p.shape[0]), i1] = -1.0
    i2 = p2.argmax(-1)
    w1 = p[np.arange(p.shape[0]), i1]
    w2 = p[np.arange(p.shape[0]), i2]
    s = w1 + w2
    return i1, i2, (w1 / s).astype(np.float32), (w2 / s).astype(np.float32)


def _tile_kxm(a2d, kouter, dtype):
    """[K, M] -> [128, K//128, M] with partition dim first."""
    k, mdim = a2d.shape
    assert k == kouter * _P
    return np.ascontiguousarray(
        a2d.reshape(kouter, _P, mdim).transpose(1, 0, 2)
    ).astype(dtype)


def _pad64(n):
    return max(256, int(math.ceil(n / 64.0)) * 64)


def _prepare(inputs):
    import ml_dtypes

    bf16 = ml_dtypes.bfloat16
    f8 = ml_dtypes.float8_e4m3

    x = np.asarray(inputs["x"], dtype=np.float32)
    w_router = np.asarray(inputs["w_router"], dtype=np.float32)
    Wg = np.asarray(inputs["Wg"], dtype=np.float32)
    Wu = np.asarray(inputs["Wu"], dtype=np.float32)
    Wd = np.asarray(inputs["Wd"], dtype=np.float32)
    sg = np.asarray(inputs["sg"], dtype=np.float32)
    su = np.asarray(inputs["su"], dtype=np.float32)
    sd = np.asarray(inputs["sd"], dtype=np.float32)

    xf = np.ascontiguousarray(x.reshape(_T, _D))
    i1, i2, w1, w2 = _route(xf, w_router)

    # Segment split per expert: A (bf16) = all rank-1 picks plus the
    # highest-weight rank-2 picks, filled to a common capacity so every
    # core does identical work; B (fp8) = the remaining low-weight
    # rank-2 picks.
    l1max = max(int((i1 == e).sum()) for e in range(_E))
    C1 = max(_pad64(l1max), 576)
    idxA, cwA, idxB, cwB = [], [], [], []
    for e in range(_E):
        ia1 = np.nonzero(i1 == e)[0]
        ia2 = np.nonzero(i2 == e)[0]
        order = ia2[np.argsort(-w2[ia2], kind="stable")]
        nfill = max(0, C1 - len(ia1))
        ia = np.concatenate([ia1, order[:nfill]])
        ib = np.sort(order[nfill:])
        wa = np.where(i1[ia] == e, w1[ia], w2[ia]).astype(np.float32)
        wb = w2[ib].astype(np.float32)
        idxA.append(ia)
        cwA.append(wa)
        idxB.append(ib)
        cwB.append(wb)
    C2 = max(384, _pad64(max(len(i) for i in idxB)))
    C = C1 + C2

    xt_f = np.ascontiguousarray(xf.T)  # [D, T]

    in_maps = []
    for e in range(_E):
        ia, wa = idxA[e], cwA[e]
        ib, wb = idxB[e], cwB[e]
        na, nb = len(ia), len(ib)

        xe1_h = np.zeros((_P, _DK, C1), bf16)
        if na:
            xe1_h[:, :, :na] = _tile_kxm(np.ascontiguousarray(xf[ia].T), _DK, bf16)

        # fp8 segment: per-tensor scales for x and the three projections.
        xb = xf[ib] if nb else np.zeros((1, _D), np.float32)
        sx = 240.0 / max(np.abs(xb).max(), 1e-30)
        swg = 240.0 / max(np.abs(Wg[e]).max(), 1e-30)
        swu = 240.0 / max(np.abs(Wu[e]).max(), 1e-30)
        swd = 240.0 / max(np.abs(Wd[e]).max(), 1e-30)
        xe2_h = np.zeros((_P, _DK, C2), f8)
        if nb:
            xq = np.clip(xf[ib] * sx, -240, 240)
            xe2_h[:, :, :nb] = _tile_kxm(np.ascontiguousarray(xq.T), _DK, f8)

        # Exact absmax of the fp8 segment's gated activation so the
        # on-device fp8 quantize of g (folded into cw) cannot overflow.
        if nb:
            gg = xb @ Wg[e].T
            uu = xb @ Wu[e].T
            hseg = (gg / (1.0 + np.exp(-gg))) * uu * wb[:, None]
            am = max(float(np.abs(hseg).max()), 1e-30)
        else:
            am = 1.0
        s_g = 240.0 / (1.3 * am)

        cw_h = np.zeros((_P, C), np.float32)
        cw_h[:, :na] = wa[None, :]
        cw_h[:, C1 : C1 + nb] = (wb * s_g / (sx * swu))[None, :]
        scal_h = np.tile(
            np.array([1.0 / (sx * swg), 1.0 / (s_g * swd)], np.float32), (_P, 1)
        )

        wgT = np.ascontiguousarray(Wg[e].T)  # [D, H]
        wg_h = np.ascontiguousarray(
            wgT.reshape(_DK, _P, _HT, _P).transpose(2, 1, 0, 3)
        ).astype(bf16)
        wuT = np.ascontiguousarray(Wu[e].T)
        wu_h = np.ascontiguousarray(
            wuT.reshape(_DK, _P, _HT, _P).transpose(2, 1, 0, 3)
        ).astype(bf16)
        wg8_h = np.ascontiguousarray(
            np.clip(wgT * swg, -240, 240)
            .reshape(_DK, _P, _HT, _P)
            .transpose(2, 1, 0, 3)
        ).astype(f8)
        wu8_h = np.ascontiguousarray(
            np.clip(wuT * swu, -240, 240)
            .reshape(_DK, _P, _HT, _P)
            .transpose(2, 1, 0, 3)
        ).astype(f8)
        wdT = np.ascontiguousarray(Wd[e].T)  # [H, D]
        wd_h = np.ascontiguousarray(
            wdT.reshape(_HT, _P, _DK, _P).transpose(2, 1, 0, 3)
        ).astype(bf16)
        wd8_h = np.ascontiguousarray(
            np.clip(wdT * swd, -240, 240)
            .reshape(_HT, _P, _DK, _P)
            .transpose(2, 1, 0, 3)
        ).astype(f8)

        # shared expert shard: token slice e%4, H-half e//4
        tsl = slice((e % 4) * _TS, (e % 4 + 1) * _TS)
        hsl = slice((e // 4) * _HH, (e // 4 + 1) * _HH)
        xt_h = _tile_kxm(np.ascontiguousarray(xt_f[:, tsl]), _DK, bf16)
        sg_h = _tile_kxm(np.ascontiguousarray(sg[hsl].T), _DK, bf16)
        su_h = _tile_kxm(np.ascontiguousarray(su[hsl].T), _DK, bf16)
        sd_h = _tile_kxm(np.ascontiguousarray(sd[:, hsl].T), _SHT, bf16)

        in_maps.append(
            {
                "xe1": xe1_h,
                "xe2": xe2_h,
                "cw": cw_h,
                "scal": scal_h,
                "wg1": wg_h,
                "wu1": wu_h,
                "wg8": wg8_h,
                "wu8": wu8_h,
                "wd1": wd_h,
                "wd8": wd8_h,
                "xt": xt_h,
                "sgh": sg_h,
                "suh": su_h,
                "sdh": sd_h,
            }
        )
    return in_maps, (idxA, idxB, C1, C2), (C1, C2)


def _combine(results, meta):
    idxA, idxB, C1, C2 = meta
    out = np.zeros((_D, _T), np.float32)
    for e in range(_E):
        ro = results[e]["rout"].astype(np.float32).reshape(_D, C1 + C2)
        sh = results[e]["shout"].astype(np.float32).reshape(_D, _TS)
        tsl = slice((e % 4) * _TS, (e % 4 + 1) * _TS)
        out[:, tsl] += sh
        if len(idxA[e]):
            out[:, idxA[e]] += ro[:, : len(idxA[e])]
        if len(idxB[e]):
            out[:, idxB[e]] += ro[:, C1 : C1 + len(idxB[e])]
    return np.ascontiguousarray(out.T).reshape(_B, _S, _D).astype(np.float32)


def kernel(**inputs):
    from concourse import bass_utils

    in_maps, meta, caps = _prepare(inputs)
    nc = _build(*caps)
    res = bass_utils.run_bass_kernel_spmd(nc, in_maps, core_ids=list(range(_NC)))
    return _combine(res.results, meta)


# revision 39
# speedup vs baseline: 1.1716x; 1.0098x over previous
"""Trainium2 Bass kernel for nn_MoEFFNBlock (B=2,S=2048,D=1024,H=2048,E=8,K=2).

Strategy (expert-parallel, 8 cores):
  host: fp32 router (matches the jax reference selection). Per expert,
        tokens split into segment A (bf16): all rank-1 picks plus the
        highest-combine-weight rank-2 picks, filled to a common
        capacity C1 so every core does identical work; and segment B
        (fp8 e4m3): the remaining low-weight rank-2 picks (weight
        <= ~0.45, so fp8's ~4% path error contributes ~1.3% to the
        output norm). Normalized combine weights and all fp8 descales
        are folded into the device-side up-projection multiply.
  core e: SwiGLU FFN for expert e: segment A in bf16 matmuls, segment
        B entirely in fp8 DoubleRow matmuls (2x PE rate; g quantized
        to fp8 on device with a host-verified scale). Plus a
        (tokens/4, H/2) shard of the shared expert: core c handles
        token slice c%4 and H-half c//4, all bf16.
  host: scatter-add segment outputs, sum the two H-half partials of
        the shared expert.

All matmul operands are pre-tiled on host so every DMA is a large
contiguous transfer, issued on one FIFO ring in consumption order with
bulk loads interleaved between weight tiles. Chunk widths stay >= 256
columns so the per-matmul LDWEIGHTS (97ns bf16 / 134ns fp8-DoubleRow)
hides under the matmul stream.
"""

import json
import math

import numpy as np

_B, _S, _D, _H, _E = 2, 2048, 1024, 2048, 8
_T = _B * _S
_P = 128
_NC = 8
_DK = _D // _P  # 8 contraction tiles over D
_HT = _H // _P  # 16 tiles over H
_HH = _H // 2  # shared-expert H columns per core (h-half)
_SHT = _HH // _P  # 8 h-tiles per core for the shared expert
_TS = _T // 4  # shared-expert token slice per core (1024)
_TC = 512  # shared-expert token chunk

_TPB_ENGINES = {"PE", "Activation", "DVE", "Pool", "SP"}


def _split_waits(bir_bytes: bytes) -> bytes:
    """walrus in this container accepts only one sync-wait per TPB
    instruction; Tile's tail drain carries several. Hoist extras onto
    NoOps that run just before the instruction on the same engine."""
    m = json.loads(bir_bytes)
    ctr = 0
    for f in m["functions"]:
        blocks = f["blocks"]
        items = blocks.items() if isinstance(blocks, dict) else enumerate(blocks)
        for _bname, bb in items:
            new_insts = []
            for inst in bb["instructions"]:
                si = inst.get("sync_info") or {}
                ow = si.get("on_wait") or []
                if len(ow) > 1 and inst.get("engine") in _TPB_ENGINES:
                    for w in ow[:-1]:
                        ctr += 1
                        nop = {
                            "name": f"I-waitsplit-{ctr}",
                            "engine": inst["engine"],
                            "opcode": "NoOp",
                            "ins": [],
                            "outs": [],
                            "sync_info": {"on_wait": [w], "on_update": []},
                        }
                        if "debug" in inst:
                            nop["debug"] = inst["debug"]
                        new_insts.append(nop)
                    si["on_wait"] = [ow[-1]]
                new_insts.append(inst)
            bb["instructions"] = new_insts
    return json.dumps(m).encode()


def _chunks(C):
    """Column chunks, each 256..512 wide, smallest first (so the first
    chunk's DMA lands soonest at startup). C must be a multiple of 64,
    C >= 256."""
    assert C >= 256 and C % 64 == 0
    out = []
    rem = C
    while rem > 512:
        w = 512 if rem >= 768 else rem - 256
        out.append(w)
        rem -= w
    out.append(rem)
    ccs, o = [], 0
    for w in sorted(out):
        assert 256 <= w <= 512
        ccs.append((o, w))
        o += w
    assert o == C
    return ccs


def _build(C1, C2):
    import concourse.bass as bass
    import concourse.mybir as mybir
    import concourse.tile as tile

    f32 = mybir.dt.float32
    bf16 = mybir.dt.bfloat16
    f8e4 = mybir.dt.float8e4
    Silu = mybir.ActivationFunctionType.Silu
    Copy = mybir.ActivationFunctionType.Copy
    mult = mybir.AluOpType.mult
    DR = mybir.MatmulPerfMode.DoubleRow

    C = C1 + C2

    nc = bass.Bass(trn_type="TRN2")
    xe1 = nc.dram_tensor("xe1", [_P, _DK, C1], bf16, kind="ExternalInput")
    xe2 = nc.dram_tensor("xe2", [_P, _DK, C2], f8e4, kind="ExternalInput")
    cw = nc.dram_tensor("cw", [_P, C], f32, kind="ExternalInput")
    scal = nc.dram_tensor("scal", [_P, 2], f32, kind="ExternalInput")
    wgu1 = nc.dram_tensor("wgu1", [_HT, _P, 2, _DK, _P], bf16, kind="ExternalInput")
    wgu8 = nc.dram_tensor("wgu8", [_HT, _P, 2, _DK, _P], f8e4, kind="ExternalInput")
    wd1 = nc.dram_tensor("wd1", [_DK, _P, _HT, _P], bf16, kind="ExternalInput")
    wd8 = nc.dram_tensor("wd8", [_DK, _P, _HT, _P], f8e4, kind="ExternalInput")
    xt = nc.dram_tensor("xt", [_P, _DK, _TS], bf16, kind="ExternalInput")
    sgh = nc.dram_tensor("sgh", [_P, _DK, _HH], bf16, kind="ExternalInput")
    suh = nc.dram_tensor("suh", [_P, _DK, _HH], bf16, kind="ExternalInput")
    sdh = nc.dram_tensor("sdh", [_P, _SHT, _D], bf16, kind="ExternalInput")
    rout = nc.dram_tensor("rout", [_DK, _P, C], bf16, kind="ExternalOutput")
    shout = nc.dram_tensor("shout", [_DK, _P, _TS], bf16, kind="ExternalOutput")

    ccs1 = _chunks(C1)

    with tile.TileContext(nc) as tc:
        with (
            tc.tile_pool(name="tmp", bufs=2) as tmp,
            tc.tile_pool(name="ps", bufs=2, space="PSUM") as psp,
            tc.tile_pool(name="bigS", bufs=1) as bigS,
            tc.tile_pool(name="cwg", bufs=1) as cwg,
            tc.tile_pool(name="strDW", bufs=3) as strDW,
        ):
            # PE warmup: dummy matmuls so the PE p-state ramps while the
            # initial DMAs are in flight.
            wtile = cwg.tile([_P, 512], bf16, name="wtile")
            nc.vector.memset(wtile[:], 0.0)
            wps = psp.tile([_P, 512], f32, tag="out", name="ops", bufs=4)
            for i in range(8):
                nc.tensor.matmul(
                    wps[:],
                    wtile[:, :_P],
                    wtile[:],
                    start=(i == 0),
                    stop=(i == 7),
                )

            cw_sb = cwg.tile([_P, C], f32, name="cw_sb")
            scal_sb = cwg.tile([_P, 2], f32, name="scal_sb")
            g_sb = cwg.tile([_P, _HT, C1], bf16, name="g_sb")
            g8_sb = cwg.tile([_P, _HT, C2], f8e4, name="g8_sb")
            sg_sb = bigS.tile([_P, _DK, _HH], bf16, name="sg_sb")
            su_sb = bigS.tile([_P, _DK, _HH], bf16, name="su_sb")
            sd_sb = bigS.tile([_P, _SHT, _D], bf16, name="sd_sb")

            # ---------- segment A (bf16) stage 1 -------------------------
            with (
                tc.tile_pool(name="poolXE", bufs=1) as poolXE,
                tc.tile_pool(name="strGU", bufs=3) as strGU,
                tc.tile_pool(name="strGU8", bufs=3) as strGU8,
            ):
                # First-needed data first: xe1 chunk 0 and the first weight
                # tiles, then the rest.
                # Startup-critical order: first (smallest) xe1 chunk and the
                # ht0 weight tiles lead; the rest interleaves with the early
                # weight stream so neither starves.
                xe1_sb = poolXE.tile([_P, _DK, C1], bf16, name="xe1_sb")
                xe2_sb = poolXE.tile([_P, _DK, C2], f8e4, name="xe2_sb")
                c0_, cn_ = ccs1[0]
                nc.sync.dma_start(
                    xe1_sb[:, :, c0_ : c0_ + cn_], xe1.ap()[:, :, c0_ : c0_ + cn_]
                )
                wgu_tiles = []
                for ht in range(3):
                    wgu_t = strGU.tile([_P, 2, _DK, _P], bf16, tag="wgu", name="wgu_t")
                    nc.sync.dma_start(wgu_t[:], wgu1.ap()[ht])
                    wgu_tiles.append(wgu_t)
                    if ht == 0:
                        for c0_, cn_ in ccs1[1:]:
                            nc.sync.dma_start(
                                xe1_sb[:, :, c0_ : c0_ + cn_],
                                xe1.ap()[:, :, c0_ : c0_ + cn_],
                            )
                    elif ht == 1:
                        nc.sync.dma_start(cw_sb[:], cw.ap())
                        nc.sync.dma_start(scal_sb[:], scal.ap())
                    elif ht == 2:
                        nc.sync.dma_start(xe2_sb[:], xe2.ap())

                for ht in range(_HT):
                    if ht < 3:
                        wgu_t = wgu_tiles[ht]
                    else:
                        wgu_t = strGU.tile(
                            [_P, 2, _DK, _P], bf16, tag="wgu", name="wgu_t"
                        )
                        nc.sync.dma_start(wgu_t[:], wgu1.ap()[ht])
                    for c0, cn in ccs1:
                        h1 = psp.tile([_P, 512], f32, tag="h1", name="h1ps")[:, :cn]
                        for k in range(_DK):
                            nc.tensor.matmul(
                                h1,
                                wgu_t[:, 0, k],
                                xe1_sb[:, k, c0 : c0 + cn],
                                start=(k == 0),
                                stop=(k == _DK - 1),
                            )
                        h2 = psp.tile([_P, 512], f32, tag="h2", name="h2ps")[:, :cn]
                        for k in range(_DK):
                            nc.tensor.matmul(
                                h2,
                                wgu_t[:, 1, k],
                                xe1_sb[:, k, c0 : c0 + cn],
                                start=(k == 0),
                                stop=(k == _DK - 1),
                            )
                        sl = tmp.tile([_P, 512], f32, tag="sl", name="sl_sb", bufs=3)[
                            :, :cn
                        ]
                        nc.scalar.activation(sl, h1, Silu)
                        t2 = tmp.tile([_P, 512], f32, tag="t2", name="t2_sb", bufs=3)[
                            :, :cn
                        ]
                        nc.vector.tensor_tensor(t2, h2, cw_sb[:, c0 : c0 + cn], mult)
                        nc.vector.tensor_tensor(
                            g_sb[:, ht, c0 : c0 + cn], sl, t2, mult
                        )

                # ---------- segment B (fp8 DoubleRow) stage 1 ------------
                ccs2 = _chunks(C2)
                for ht in range(_HT):
                    wgu8_t = strGU8.tile(
                        [_P, 2, _DK, _P], f8e4, tag="wgu8", name="wgu8_t"
                    )
                    nc.sync.dma_start(wgu8_t[:], wgu8.ap()[ht])
                    for c0, cn in ccs2:
                        h1 = psp.tile([_P, 512], f32, tag="h1", name="h1ps")[:, :cn]
                        for k in range(_DK // 2):
                            nc.tensor.matmul(
                                h1,
                                wgu8_t[:, 0, 2 * k : 2 * k + 2],
                                xe2_sb[:, 2 * k : 2 * k + 2, c0 : c0 + cn],
                                start=(k == 0),
                                stop=(k == _DK // 2 - 1),
                                perf_mode=DR,
                            )
                        h2 = psp.tile([_P, 512], f32, tag="h2", name="h2ps")[:, :cn]
                        for k in range(_DK // 2):
                            nc.tensor.matmul(
                                h2,
                                wgu8_t[:, 1, 2 * k : 2 * k + 2],
                                xe2_sb[:, 2 * k : 2 * k + 2, c0 : c0 + cn],
                                start=(k == 0),
                                stop=(k == _DK // 2 - 1),
                                perf_mode=DR,
                            )
                        sl = tmp.tile([_P, 512], f32, tag="sl", name="sl_sb", bufs=3)[
                            :, :cn
                        ]
                        nc.scalar.activation(sl, h1, Silu, scale=scal_sb[:, 0:1])
                        t2 = tmp.tile([_P, 512], f32, tag="t2", name="t2_sb", bufs=3)[
                            :, :cn
                        ]
                        nc.vector.tensor_tensor(
                            t2, h2, cw_sb[:, C1 + c0 : C1 + c0 + cn], mult
                        )
                        nc.vector.tensor_tensor(
                            g8_sb[:, ht, c0 : c0 + cn], sl, t2, mult
                        )

            # ---------- stage 2 (down-proj, both segments) ---------------
            with (
                tc.tile_pool(name="strS", bufs=2) as strS,
                tc.tile_pool(name="gsp", bufs=2) as gsp,
            ):
                xt_tiles = [
                    strS.tile([_P, _DK, _TC], bf16, tag="xt", name="xt_sb")
                    for _ in range(0, _TS, _TC)
                ]

                for dt_i in range(_DK):
                    wd_t = strDW.tile([_P, _HT, _P], bf16, tag="wd", name="wd_t")
                    nc.sync.dma_start(wd_t[:], wd1.ap()[dt_i])
                    wd8_t = strDW.tile([_P, _HT, _P], f8e4, tag="wd8", name="wd8_t")
                    nc.sync.dma_start(wd8_t[:], wd8.ap()[dt_i])
                    # Interleave the bulk shared-expert loads between the
                    # down-proj weight tiles so neither starves the other.
                    if dt_i == 0:
                        nc.sync.dma_start(sg_sb[:], sgh.ap())
                    elif dt_i == 2:
                        nc.sync.dma_start(su_sb[:], suh.ap())
                    elif dt_i == 4:
                        nc.sync.dma_start(sd_sb[:], sdh.ap())
                    elif dt_i == 5:
                        nc.sync.dma_start(xt_tiles[0][:], xt.ap()[:, :, 0:_TC])
                    elif dt_i == 6:
                        nc.sync.dma_start(xt_tiles[1][:], xt.ap()[:, :, _TC : 2 * _TC])
                    for c0, cn in ccs1:
                        ops = psp.tile(
                            [_P, 512], f32, tag="out", name="ops", bufs=4
                        )[:, :cn]
                        for k in range(_HT):
                            nc.tensor.matmul(
                                ops,
                                wd_t[:, k],
                                g_sb[:, k, c0 : c0 + cn],
                                start=(k == 0),
                                stop=(k == _HT - 1),
                            )
                        ro = tmp.tile(
                            [_P, 512], bf16, tag="ro", name="ro_sb", bufs=6
                        )[:, :cn]
                        nc.vector.tensor_copy(ro, ops)
                        nc.sync.dma_start(rout.ap()[dt_i][:, c0 : c0 + cn], ro)
                    # fp8 segment down-proj: DoubleRow over the 16 h-tiles,
                    # descaled on the Activation engine during the copy.
                    for c0, cn in _chunks(C2):
                        ops = psp.tile([_P, 512], f32, tag="out", name="ops", bufs=4)[
                            :, :cn
                        ]
                        for k in range(_HT // 2):
                            nc.tensor.matmul(
                                ops,
                                wd8_t[:, 2 * k : 2 * k + 2],
                                g8_sb[:, 2 * k : 2 * k + 2, c0 : c0 + cn],
                                start=(k == 0),
                                stop=(k == _HT // 2 - 1),
                                perf_mode=DR,
                            )
                        ro = tmp.tile([_P, 512], bf16, tag="ro", name="ro_sb", bufs=6)[
                            :, :cn
                        ]
                        nc.scalar.activation(ro, ops, Copy, scale=scal_sb[:, 1:2])
                        nc.sync.dma_start(
                            rout.ap()[dt_i][:, C1 + c0 : C1 + c0 + cn], ro
                        )

                # ---------- shared expert, software-pipelined ------------
                def s_hstage(t0, xt_sb):
                    gs = gsp.tile([_P, _SHT, _TC], bf16, tag="gs", name="gs_sb")
                    for hs in range(_SHT):
                        h1 = psp.tile([_P, 512], f32, tag="h1", name="h1ps")
                        for k in range(_DK):
                            nc.tensor.matmul(
                                h1,
                                sg_sb[:, k, hs * _P : (hs + 1) * _P],
                                xt_sb[:, k],
                                start=(k == 0),
                                stop=(k == _DK - 1),
                            )
                        h2 = psp.tile([_P, 512], f32, tag="h2", name="h2ps")
                        for k in range(_DK):
                            nc.tensor.matmul(
                                h2,
                                su_sb[:, k, hs * _P : (hs + 1) * _P],
                                xt_sb[:, k],
                                start=(k == 0),
                                stop=(k == _DK - 1),
                            )
                        sl = tmp.tile([_P, 512], f32, tag="sl", name="sl_sb", bufs=3)
                        nc.scalar.activation(sl, h1, Silu)
                        nc.vector.tensor_tensor(gs[:, hs], sl, h2, mult)
                    return gs

                def s_dstage(t0, gs):
                    for dt_i in range(_DK):
                        ops = psp.tile([_P, 512], f32, tag="out", name="ops", bufs=4)
                        for k in range(_SHT):
                            nc.tensor.matmul(
                                ops,
                                sd_sb[:, k, dt_i * _P : (dt_i + 1) * _P],
                                gs[:, k],
                                start=(k == 0),
                                stop=(k == _SHT - 1),
                            )
                        so = tmp.tile(
                            [_P, 512], bf16, tag="ro", name="ro_sb", bufs=6
                        )
                        if dt_i % 2:
                            nc.scalar.copy(so, ops)
                        else:
                            nc.vector.tensor_copy(so, ops)
                        nc.sync.dma_start(shout.ap()[dt_i][:, t0 : t0 + _TC], so)

                prev = None
                for ci, t0 in enumerate(range(0, _TS, _TC)):
                    gs = s_hstage(t0, xt_tiles[ci])
                    if prev is not None:
                        s_dstage(prev[0], prev[1])
                    prev = (t0, gs)
                s_dstage(prev[0], prev[1])

    orig = nc.to_json_bytes
    nc.to_json_bytes = lambda: _split_waits(orig())
    return nc


def _route(xf, w_router):
    """fp32 router matching the jax reference: softmax over logits, top-2
    (selection identical to jax.lax.top_k for non-tied logits), weights
    renormalized over the selected pair."""
    logits = xf @ w_router.T.astype(np.float32)
    m = logits.max(-1, keepdims=True)
    p = np.exp(logits - m)
    p /= p.sum(-1, keepdims=True)
    i1 = p.argmax(-1)
    p2 = p.copy()
    p2[np.arange(p.shape[0]), i1] = -1.0
    i2 = p2.argmax(-1)
    w1 = p[np.arange(p.shape[0]), i1]
    w2 = p[np.arange(p.shape[0]), i2]
    s = w1 + w2
    return i1, i2, (w1 / s).astype(np.float32), (w2 / s).astype(np.float32)


def _tile_kxm(a2d, kouter, dtype):
    """[K, M] -> [128, K//128, M] with partition dim first."""
    k, mdim = a2d.shape
    assert k == kouter * _P
    return np.ascontiguousarray(
        a2d.reshape(kouter, _P, mdim).transpose(1, 0, 2)
    ).astype(dtype)


def _pad64(n):
    return max(256, int(math.ceil(n / 64.0)) * 64)


def _prepare(inputs):
    import ml_dtypes

    bf16 = ml_dtypes.bfloat16
    f8 = ml_dtypes.float8_e4m3

    x = np.asarray(inputs["x"], dtype=np.float32)
    w_router = np.asarray(inputs["w_router"], dtype=np.float32)
    Wg = np.asarray(inputs["Wg"], dtype=np.float32)
    Wu = np.asarray(inputs["Wu"], dtype=np.float32)
    Wd = np.asarray(inputs["Wd"], dtype=np.float32)
    sg = np.asarray(inputs["sg"], dtype=np.float32)
    su = np.asarray(inputs["su"], dtype=np.float32)
    sd = np.asarray(inputs["sd"], dtype=np.float32)

    xf = np.ascontiguousarray(x.reshape(_T, _D))
    i1, i2, w1, w2 = _route(xf, w_router)

    # Segment split per expert: A (bf16) = all rank-1 picks plus the
    # highest-weight rank-2 picks, filled to a common capacity so every
    # core does identical work; B (fp8) = the remaining low-weight
    # rank-2 picks.
    l1max = max(int((i1 == e).sum()) for e in range(_E))
    C1 = max(_pad64(l1max), 576)
    idxA, cwA, idxB, cwB = [], [], [], []
    for e in range(_E):
        ia1 = np.nonzero(i1 == e)[0]
        ia2 = np.nonzero(i2 == e)[0]
        order = ia2[np.argsort(-w2[ia2], kind="stable")]
        nfill = max(0, C1 - len(ia1))
        ia = np.concatenate([ia1, order[:nfill]])
        ib = np.sort(order[nfill:])
        wa = np.where(i1[ia] == e, w1[ia], w2[ia]).astype(np.float32)
        wb = w2[ib].astype(np.float32)
        idxA.append(ia)
        cwA.append(wa)
        idxB.append(ib)
        cwB.append(wb)
    C2 = max(384, _pad64(max(len(i) for i in idxB)))
    C = C1 + C2

    xt_f = np.ascontiguousarray(xf.T)  # [D, T]

    in_maps = []
    for e in range(_E):
        ia, wa = idxA[e], cwA[e]
        ib, wb = idxB[e], cwB[e]
        na, nb = len(ia), len(ib)

        xe1_h = np.zeros((_P, _DK, C1), bf16)
        if na:
            xe1_h[:, :, :na] = _tile_kxm(np.ascontiguousarray(xf[ia].T), _DK, bf16)

        # fp8 segment: per-tensor scales for x and the three projections.
        xb = xf[ib] if nb else np.zeros((1, _D), np.float32)
        sx = 240.0 / max(np.abs(xb).max(), 1e-30)
        swg = 240.0 / max(np.abs(Wg[e]).max(), 1e-30)
        swu = 240.0 / max(np.abs(Wu[e]).max(), 1e-30)
        swd = 240.0 / max(np.abs(Wd[e]).max(), 1e-30)
        xe2_h = np.zeros((_P, _DK, C2), f8)
        if nb:
            xq = np.clip(xf[ib] * sx, -240, 240)
            xe2_h[:, :, :nb] = _tile_kxm(np.ascontiguousarray(xq.T), _DK, f8)

        # Exact absmax of the fp8 segment's gated activation so the
        # on-device fp8 quantize of g (folded into cw) cannot overflow.
        if nb:
            gg = xb @ Wg[e].T
            uu = xb @ Wu[e].T
            hseg = (gg / (1.0 + np.exp(-gg))) * uu * wb[:, None]
            am = max(float(np.abs(hseg).max()), 1e-30)
        else:
            am = 1.0
        s_g = 240.0 / (1.3 * am)

        cw_h = np.zeros((_P, C), np.float32)
        cw_h[:, :na] = wa[None, :]
        cw_h[:, C1 : C1 + nb] = (wb * s_g / (sx * swu))[None, :]
        scal_h = np.tile(
            np.array([1.0 / (sx * swg), 1.0 / (s_g * swd)], np.float32), (_P, 1)
        )

        wgT = np.ascontiguousarray(Wg[e].T)  # [D, H]
        wuT = np.ascontiguousarray(Wu[e].T)
        wg_h = wgT.reshape(_DK, _P, _HT, _P).transpose(2, 1, 0, 3)
        wu_h = wuT.reshape(_DK, _P, _HT, _P).transpose(2, 1, 0, 3)
        # gate/up tiles stacked so each h-tile streams as ONE DMA
        wgu_h = np.ascontiguousarray(np.stack([wg_h, wu_h], axis=2)).astype(bf16)
        wg8_h = (
            np.clip(wgT * swg, -240, 240)
            .reshape(_DK, _P, _HT, _P)
            .transpose(2, 1, 0, 3)
        )
        wu8_h = (
            np.clip(wuT * swu, -240, 240)
            .reshape(_DK, _P, _HT, _P)
            .transpose(2, 1, 0, 3)
        )
        wgu8_h = np.ascontiguousarray(np.stack([wg8_h, wu8_h], axis=2)).astype(f8)
        wdT = np.ascontiguousarray(Wd[e].T)  # [H, D]
        wd_h = np.ascontiguousarray(
            wdT.reshape(_HT, _P, _DK, _P).transpose(2, 1, 0, 3)
        ).astype(bf16)
        wd8_h = np.ascontiguousarray(
            np.clip(wdT * swd, -240, 240)
            .reshape(_HT, _P, _DK, _P)
            .transpose(2, 1, 0, 3)
        ).astype(f8)

        # shared expert shard: token slice e%4, H-half e//4
        tsl = slice((e % 4) * _TS, (e % 4 + 1) * _TS)
        hsl = slice((e // 4) * _HH, (e // 4 + 1) * _HH)
        xt_h = _tile_kxm(np.ascontiguousarray(xt_f[:, tsl]), _DK, bf16)
        sg_h = _tile_kxm(np.ascontiguousarray(sg[hsl].T), _DK, bf16)
        su_h = _tile_kxm(np.ascontiguousarray(su[hsl].T), _DK, bf16)
        sd_h = _tile_kxm(np.ascontiguousarray(sd[:, hsl].T), _SHT, bf16)

        in_maps.append(
            {
                "xe1": xe1_h,
                "xe2": xe2_h,
                "cw": cw_h,
                "scal": scal_h,
                "wgu1": wgu_h,
                "wgu8": wgu8_h,
                "wd1": wd_h,
                "wd8": wd8_h,
                "xt": xt_h,
                "sgh": sg_h,
                "suh": su_h,
                "sdh": sd_h,
            }
        )
    return in_maps, (idxA, idxB, C1, C2), (C1, C2)


def _combine(results, meta):
    idxA, idxB, C1, C2 = meta
    out = np.zeros((_D, _T), np.float32)
    for e in range(_E):
        ro = results[e]["rout"].astype(np.float32).reshape(_D, C1 + C2)
        sh = results[e]["shout"].astype(np.float32).reshape(_D, _TS)
        tsl = slice((e % 4) * _TS, (e % 4 + 1) * _TS)
        out[:, tsl] += sh
        if len(idxA[e]):
            out[:, idxA[e]] += ro[:, : len(idxA[e])]
        if len(idxB[e]):
            out[:, idxB[e]] += ro[:, C1 : C1 + len(idxB[e])]
    return np.ascontiguousarray(out.T).reshape(_B, _S, _D).astype(np.float32)


def kernel(**inputs):
    from concourse import bass_utils

    in_maps, meta, caps = _prepare(inputs)
    nc = _build(*caps)
    res = bass_utils.run_bass_kernel_spmd(nc, in_maps, core_ids=list(range(_NC)))
    return _combine(res.results, meta)
